# revision 5
# baseline (speedup 1.0000x reference)
"""Class-based decoder (MoE-style routing) on 8 trn2 NeuronCores.

Strategy: expert-parallel. Classes are padded 250->256 and split 32 per core.
On the host, tokens are grouped by class into capacity-padded slots (C tokens
per class slot, C in {32,64,128}); class slots that overflow C spill into
extra slots holding a duplicate of the class weights.  Each core receives:
  - xT   [128, n_mt*KCH*128]   its padded tokens, pre-transposed k-major
  - wcT  [128, KCH*NCLS_P]     the (replicated) class-decoder weights, k-major
  - wwT  [n_grp, 128, GRP*KCH*CHUNK]  its word-decoder shard, k-major, grouped
and computes, for every 128-token tile, the class logits (x @ Wc.T) and the
per-class word logits (x_c @ Ww[c].T) as PE matmuls accumulating K=512 over
4 PSUM chunks.  Class slots of a tile are col-tiled into one PSUM tile.
Biases (zero in practice, but handled for correctness) are added on the host
during the final unpermute.

Precision modes:
  f32  : exact fp32 matmuls (2-pass LOW/HIGH on PE; bit-exact, slowest)
  f32r : fp32 data, single-pass PE (TF32-like rounding). Classes are paired
         into N=400 matmuls and NCLS padded to 256 so the free dim is >=256,
         where f32r streams at full rate.
  bf16 : weights/activations cast to bf16 (halves the big W DMA)
  fp8  : word-decoder weights in float8e3 (E3M4), prescaled by 128 so the
         uniform(-0.1,0.1) values land in the normal range; x/Wc stay bf16
         (PE allows mixed non-fp32 operand dtypes) and the output is stored
         bf16.  Halves the dominant W DMA again vs bf16; ~0.9% rel err.
"""

import numpy as np
from contextlib import ExitStack

import concourse.bass as bass
import concourse.bacc as bacc
import concourse.tile as tile
import concourse.mybir as mybir
from concourse.bass_utils import run_bass_kernel_spmd

NHID = 512
NCLS = 250
CHUNK = 200
NCORES = 8
KCH = NHID // 128          # 4 contraction chunks of 128
NCLS_PAD = 256             # classes padded so each core owns an equal shard
CPC = NCLS_PAD // NCORES   # classes per core
NCOL = NCLS + CHUNK        # 450 output columns
F32 = mybir.dt.float32

MODE = "fp8"               # default precision mode; see module docstring
WSCALE = 128.0             # fp8 mode: host prescale of Ww (undone after run)

LAST_RESULT = None         # BassKernelResults of the most recent device run
_program_cache = {}

_MM_DT = {"f32": mybir.dt.float32, "f32r": mybir.dt.float32r,
          "bf16": mybir.dt.bfloat16, "fp8": mybir.dt.bfloat16}
_NP_DT = {"f32": np.float32, "f32r": np.float32, "bf16": None, "fp8": None}
try:
    import ml_dtypes
    _NP_DT["bf16"] = ml_dtypes.bfloat16
    _NP_DT["fp8"] = ml_dtypes.bfloat16   # x/Wc dtype; Ww uses float8_e3m4
except ImportError:
    pass


def _build_program(C, slots, mode):
    """One SPMD program: slots class-slots of C tokens each, per core.

    f32 uses the "coltile" scheme: per class slot, an M=C matmul col-tiled
    into a shared PSUM tile (exact 2-pass fp32).
    f32r/bf16 use the "block" scheme: every matmul is M=128 (all slots of an
    m-tile), and the word logits come as per_mt//2 halves of N=2*CHUNK whose
    off-diagonal class blocks are discarded by the PSUM->SBUF copies.  This
    keeps N>=256 (full-rate f32r) and NumWeights=128 (FWL weight loads).
    """
    n_mt = (slots * C) // 128  # 128-token m-tiles
    npad = slots * C
    per_mt = 128 // C          # class slots per m-tile
    block = mode in ("f32r", "bf16", "fp8")
    fp8 = mode == "fp8"
    # class slots per pw matmul and word-columns per pw matmul
    gs = 2 if (block and per_mt >= 2) else 1
    gw = gs * CHUNK            # 400 paired / 200 single
    n_half = per_mt // gs      # pw matmul groups per m-tile
    ncls_p = 256 if block else NCLS  # N>=256 keeps f32r at full rate
    # C=16 diag copies would need 16-partition bases (illegal); store each
    # 32-row band's full pair block instead and let the host pick the diagonal
    wide = block and C == 16
    ocol = NCLS + (gw if wide else CHUNK)
    dt = _MM_DT[mode]
    dt_w = mybir.dt.float8e3 if fp8 else dt
    dt_o = mybir.dt.bfloat16 if fp8 else F32

    nc = bacc.Bacc("TRN2", target_bir_lowering=False, debug=False,
                   num_devices=NCORES)
    xT = nc.dram_tensor("xT", [128, n_mt * KCH * 128], dt, kind="ExternalInput")
    wcT = nc.dram_tensor("wcT", [128, KCH * ncls_p], dt, kind="ExternalInput")
    # W groups: one DMA per m-tile worth of class slots
    wwT = nc.dram_tensor("wwT", [n_mt, 128, per_mt * KCH * CHUNK], dt_w,
                         kind="ExternalInput")
    out = nc.dram_tensor("out", [npad, ocol], dt_o, kind="ExternalOutput")

    with tile.TileContext(nc) as tc, ExitStack() as ctx:
        xpool = ctx.enter_context(tc.tile_pool(name="x", bufs=1))
        wcpool = ctx.enter_context(tc.tile_pool(name="wc", bufs=1))
        wpool = ctx.enter_context(tc.tile_pool(name="w", bufs=12))
        opool = ctx.enter_context(tc.tile_pool(name="o", bufs=8))
        pcp = ctx.enter_context(
            tc.tile_pool(name="pc", bufs=2, space=bass.MemorySpace.PSUM))
        pwp = ctx.enter_context(
            tc.tile_pool(name="pw", bufs=6, space=bass.MemorySpace.PSUM))

        # three independent DMA streams so nothing blocks the W firehose:
        #   sync (SP HWDGE): only the big W chunks, back to back
        #   scalar (ACT HWDGE): wc + per-m-tile x loads
        #   gpsimd (SWDGE): output stores
        wc_sb = wcpool.tile([128, KCH * ncls_p], dt)
        x_sb = xpool.tile([128, n_mt * KCH * 128], dt)
        if fp8:
            # one big x DMA upfront (x is small; W chunks then own the queues)
            nc.scalar.dma_start(x_sb[:], xT[:])
        nc.scalar.dma_start(wc_sb[:], wcT[:])

        wchunk = KCH * gw  # free-dim elems per W DMA (one pw matmul group)
        for m in range(n_mt):
            if not fp8:
                # x columns for this m-tile: [(m*KCH+j)*128 + t]
                nc.scalar.dma_start(x_sb[:, m * KCH * 128:(m + 1) * KCH * 128],
                                    xT[:, m * KCH * 128:(m + 1) * KCH * 128])

            def xcol(j, lo, hi):
                base = (m * KCH + j) * 128
                return x_sb[:, base + lo:base + hi]

            # class logits for these 128 tokens
            pc_ps = pcp.tile([128, ncls_p], F32)
            for j in range(KCH):
                nc.tensor.matmul(
                    pc_ps[:, :],
                    xcol(j, 0, 128),
                    wc_sb[:, j * ncls_p:(j + 1) * ncls_p],
                    start=(j == 0), stop=(j == KCH - 1),
                )

            o_sb = opool.tile([128, ocol], F32)
            nc.vector.tensor_copy(o_sb[:, :NCLS], pc_ps[:, :NCLS])

            if block:
                # word logits: per half, one M=128 matmul of N=gw covering
                # gs classes; only each slot's own class block is kept
                for h in range(n_half):
                    w_sb = wpool.tile([128, wchunk], dt, tag="w")
                    weng = nc.sync if (m * n_half + h) % 2 == 0 else nc.scalar
                    weng.dma_start(
                        w_sb[:], wwT[m][:, h * wchunk:(h + 1) * wchunk])
                    pw_ps = pwp.tile([128, gw], F32, tag="pw")
                    for j in range(KCH):
                        nc.tensor.matmul(
                            pw_ps[:, :],
                            xcol(j, 0, 128),
                            w_sb[:, j * gw:(j + 1) * gw],
                            start=(j == 0), stop=(j == KCH - 1),
                        )
                    if wide:
                        b = gs * C  # 32-row band of this pair
                        nc.vector.tensor_copy(
                            o_sb[h * b:(h + 1) * b, NCLS:],
                            pw_ps[h * b:(h + 1) * b, :])
                    else:
                        for a in range(gs):
                            q = h * gs + a  # slot in m-tile
                            nc.vector.tensor_copy(
                                o_sb[q * C:(q + 1) * C, NCLS:],
                                pw_ps[q * C:(q + 1) * C,
                                      a * CHUNK:(a + 1) * CHUNK])
            else:
                # exact f32: per-slot M=C matmuls col-tiled into one tile
                w_sb = wpool.tile([128, per_mt * KCH * CHUNK], dt, tag="w")
                nc.sync.dma_start(w_sb[:], wwT[m])
                pw_ps = pwp.tile([128, CHUNK], F32, tag="pw")
                for q in range(per_mt):
                    for j in range(KCH):
                        nc.tensor.matmul(
                            pw_ps[q * C:(q + 1) * C, :],
                            xcol(j, q * C, (q + 1) * C),
                            w_sb[:, (q * KCH + j) * CHUNK:
                                 (q * KCH + j + 1) * CHUNK],
                            start=(j == 0), stop=(j == KCH - 1),
                            tile_position=(0, q * C),
                        )
                nc.vector.tensor_copy(o_sb[:, NCLS:], pw_ps[:])

            nc.gpsimd.dma_start(out[m * 128:(m + 1) * 128, :], o_sb[:])

    nc.compile()
    return nc


def _route(cls, mode):
    """Group tokens by class into capacity-padded slots: one slot per class,
    C tokens of capacity.  The (rare) tokens beyond a class's capacity are
    returned as `overflow` and evaluated directly on the host in numpy.

    Returns (C, slots, tok_idx [NCORES, slots*C] int64 token id or -1,
    slot_cls [NCORES, slots] class id per slot, overflow token-id array).
    """
    counts = np.bincount(cls, minlength=NCLS_PAD)
    # coltile (exact f32) needs C to be a multiple of 32 for PSUM col tiling
    cands = (16, 32, 64, 128) if mode in ("f32r", "bf16") else (32, 64, 128)
    C = cands[-1]
    for c in cands:
        if int(np.maximum(counts - c, 0).sum()) <= 32:
            C = c
            break

    order = np.argsort(cls, kind="stable")
    starts = np.zeros(NCLS_PAD + 1, np.int64)
    starts[1:] = np.cumsum(counts)

    slots = CPC  # one slot per class owned by the core
    tok_idx = np.full((NCORES, slots * C), -1, np.int64)
    slot_cls = np.full((NCORES, slots), -1, np.int64)
    overflow = []
    for k in range(NCORES):
        for s in range(slots):
            c = k * CPC + s
            lo, cnt = int(starts[c]), int(counts[c])
            n = min(C, cnt)
            slot_cls[k, s] = c
            if n > 0:
                tok_idx[k, s * C:s * C + n] = order[lo:lo + n]
            if cnt > C:
                overflow.append(order[lo + C:lo + cnt])
    overflow = (np.concatenate(overflow) if overflow
                else np.zeros((0,), np.int64))
    return C, slots, tok_idx, slot_cls, overflow


def kernel(x, Wc, bc, Ww, bw, cls_idx, _trace=False, _trace_cores=None,
           _mode=None):
    global LAST_RESULT
    mode = _mode or MODE
    ndt = _NP_DT[mode]
    if ndt is None:
        mode = "f32"
        ndt = np.float32

    x = np.ascontiguousarray(np.asarray(x, np.float32))
    Wc = np.ascontiguousarray(np.asarray(Wc, np.float32))
    bc = np.asarray(bc, np.float32)
    Ww = np.ascontiguousarray(np.asarray(Ww, np.float32))
    bw = np.asarray(bw, np.float32)
    cls = np.asarray(cls_idx).astype(np.int64).ravel()
    N = cls.shape[0]

    C, slots, tok_idx, slot_cls, overflow = _route(cls, mode)
    npad = slots * C
    n_mt = npad // 128
    per_mt = 128 // C
    block = mode in ("f32r", "bf16")
    gs = 2 if (block and per_mt >= 2) else 1
    ncls_p = 256 if block else NCLS

    key = (C, slots, mode)
    if key not in _program_cache:
        _program_cache[key] = _build_program(C, slots, mode)
    nc = _program_cache[key]

    # wcT [128, KCH*ncls_p]: wcT[p, j*ncls_p+c] = Wc[c, j*128+p]  (replicated)
    Wc_p = Wc if ncls_p == NCLS else np.concatenate(
        [Wc, np.zeros((ncls_p - NCLS, NHID), np.float32)], 0)
    wcT = np.ascontiguousarray(
        Wc_p.reshape(ncls_p, KCH, 128).transpose(2, 1, 0)
            .reshape(128, KCH * ncls_p).astype(ndt))

    Ww_pad = np.zeros((NCLS_PAD, CHUNK, NHID), np.float32)
    Ww_pad[:NCLS] = Ww

    in_maps = []
    for k in range(NCORES):
        # per-slot k-major weights: tmp[s, j, p, w] = Ww[cls_s, w, j*128+p]
        wsel = Ww_pad[np.maximum(slot_cls[k], 0)]
        wsel[slot_cls[k] < 0] = 0.0
        tmp = wsel.reshape(slots, CHUNK, KCH, 128).transpose(0, 2, 3, 1)
        if gs == 2:
            # group = m-tile (per_mt slots); within: pair r, then j, then
            # the two slots' CHUNK columns side by side
            tmp = tmp.reshape(n_mt, per_mt // 2, 2, KCH, 128, CHUNK)
            tmp = tmp.transpose(0, 4, 1, 3, 2, 5)  # [n_mt,128,pair,j,2,CHUNK]
        else:
            tmp = tmp.reshape(n_mt, per_mt, KCH, 128, CHUNK)
            tmp = tmp.transpose(0, 3, 1, 2, 4)     # [n_mt,128,q,j,CHUNK]
        wwT = np.ascontiguousarray(
            tmp.reshape(n_mt, 128, per_mt * KCH * CHUNK).astype(ndt))

        ti = tok_idx[k]
        xk = x[np.maximum(ti, 0)]
        xk[ti < 0] = 0.0
        # xT[p, (m*KCH+j)*128 + t] = xk[m*128+t, j*128+p]
        xT = np.ascontiguousarray(
            xk.reshape(n_mt, 128, KCH, 128).transpose(3, 0, 2, 1)
              .reshape(128, n_mt * KCH * 128).astype(ndt))
        in_maps.append({"xT": xT, "wcT": wcT, "wwT": wwT})

    LAST_RESULT = run_bass_kernel_spmd(
        nc, in_maps, list(range(NCORES)), trace=_trace,
        trace_cores=(_trace_cores if _trace else None))

    wide = block and C == 16
    out = np.zeros((N, NCOL), np.float32)
    if wide:
        # row r of a core's output holds its pair's full 2*CHUNK block;
        # slot parity selects which CHUNK half is this row's class
        a_row = (np.arange(npad) // C) % 2
    for k in range(NCORES):
        ok = np.asarray(LAST_RESULT.results[k]["out"], np.float32)
        if wide:
            words = np.where((a_row == 0)[:, None],
                             ok[:, NCLS:NCLS + CHUNK],
                             ok[:, NCLS + CHUNK:NCLS + 2 * CHUNK])
            ok = np.concatenate([ok[:, :NCLS], words], 1)
        valid = tok_idx[k] >= 0
        out[tok_idx[k][valid]] = ok[valid]

    if overflow.size:
        # rare capacity-overflow tokens: evaluate directly on the host
        xo = x[overflow]                                   # [no, NHID]
        out[overflow, :NCLS] = xo @ Wc.T
        co = cls[overflow]
        out[overflow, NCLS:] = np.einsum(
            "nkh,nh->nk", Ww[co], xo, optimize=True)

    out[:, :NCLS] += bc
    out[:, NCLS:] += bw[cls]
    return out



# revision 12
# speedup vs baseline: 1.0514x; 1.0514x over previous
"""Class-based decoder (MoE-style routing) on 8 trn2 NeuronCores.

Strategy: expert-parallel. Classes are padded 250->256 and split 32 per core.
On the host, tokens are grouped by class into capacity-padded slots (C tokens
per class slot, C in {32,64,128}); class slots that overflow C spill into
extra slots holding a duplicate of the class weights.  Each core receives:
  - xT   [128, n_mt*KCH*128]   its padded tokens, pre-transposed k-major
  - wcT  [128, KCH*NCLS_P]     the (replicated) class-decoder weights, k-major
  - wwT  [n_grp, 128, GRP*KCH*CHUNK]  its word-decoder shard, k-major, grouped
and computes, for every 128-token tile, the class logits (x @ Wc.T) and the
per-class word logits (x_c @ Ww[c].T) as PE matmuls accumulating K=512 over
4 PSUM chunks.  Class slots of a tile are col-tiled into one PSUM tile.
Biases (zero in practice, but handled for correctness) are added on the host
during the final unpermute.

Precision modes:
  f32  : exact fp32 matmuls (2-pass LOW/HIGH on PE; bit-exact, slowest)
  f32r : fp32 data, single-pass PE (TF32-like rounding). Classes are paired
         into N=400 matmuls and NCLS padded to 256 so the free dim is >=256,
         where f32r streams at full rate.
  bf16 : weights/activations cast to bf16 (halves the big W DMA)
  fp8  : word-decoder weights in float8e3 (E3M4), prescaled by 128 so the
         uniform(-0.1,0.1) values land in the normal range; x/Wc stay bf16
         (PE allows mixed non-fp32 operand dtypes) and the output is stored
         bf16.  Halves the dominant W DMA again vs bf16; ~0.9% rel err.
"""

import numpy as np
from contextlib import ExitStack

import concourse.bass as bass
import concourse.bacc as bacc
import concourse.tile as tile
import concourse.mybir as mybir
from concourse.bass_utils import run_bass_kernel_spmd

NHID = 512
NCLS = 250
CHUNK = 200
NCORES = 8
KCH = NHID // 128          # 4 contraction chunks of 128
NCLS_PAD = 256             # classes padded so each core owns an equal shard
CPC = NCLS_PAD // NCORES   # classes per core
NCOL = NCLS + CHUNK        # 450 output columns
F32 = mybir.dt.float32

MODE = "fp8"               # default precision mode; see module docstring
WSCALE = 128.0             # fp8 mode: host prescale of Ww (undone after run)

LAST_RESULT = None         # BassKernelResults of the most recent device run
_program_cache = {}

_MM_DT = {"f32": mybir.dt.float32, "f32r": mybir.dt.float32r,
          "bf16": mybir.dt.bfloat16, "fp8": mybir.dt.bfloat16}
_NP_DT = {"f32": np.float32, "f32r": np.float32, "bf16": None, "fp8": None}
try:
    import ml_dtypes
    _NP_DT["bf16"] = ml_dtypes.bfloat16
    _NP_DT["fp8"] = ml_dtypes.bfloat16   # x/Wc dtype; Ww uses float8_e3m4
except ImportError:
    pass


def _build_program(C, slots, mode):
    """One SPMD program: slots class-slots of C tokens each, per core.

    f32 uses the "coltile" scheme: per class slot, an M=C matmul col-tiled
    into a shared PSUM tile (exact 2-pass fp32).
    f32r/bf16 use the "block" scheme: every matmul is M=128 (all slots of an
    m-tile), and the word logits come as per_mt//2 halves of N=2*CHUNK whose
    off-diagonal class blocks are discarded by the PSUM->SBUF copies.  This
    keeps N>=256 (full-rate f32r) and NumWeights=128 (FWL weight loads).
    """
    n_mt = (slots * C) // 128  # 128-token m-tiles
    npad = slots * C
    per_mt = 128 // C          # class slots per m-tile
    block = mode in ("f32r", "bf16", "fp8")
    fp8 = mode == "fp8"
    # class slots per pw matmul and word-columns per pw matmul
    gs = 2 if (block and per_mt >= 2) else 1
    gw = gs * CHUNK            # 400 paired / 200 single
    n_half = per_mt // gs      # pw matmul groups per m-tile
    ncls_p = 256 if block else NCLS  # N>=256 keeps f32r at full rate
    # C=16 diag copies would need 16-partition bases (illegal); store each
    # 32-row band's full pair block instead and let the host pick the diagonal
    wide = block and C == 16
    ocol = NCLS + (gw if wide else CHUNK)
    dt = _MM_DT[mode]
    dt_w = mybir.dt.float8e3 if fp8 else dt
    dt_o = mybir.dt.bfloat16 if fp8 else F32

    nc = bacc.Bacc("TRN2", target_bir_lowering=False, debug=False,
                   num_devices=NCORES)
    xT = nc.dram_tensor("xT", [128, n_mt * KCH * 128], dt, kind="ExternalInput")
    wcT = nc.dram_tensor("wcT", [128, KCH * ncls_p], dt, kind="ExternalInput")
    # W groups: one DMA per m-tile worth of class slots
    wwT = nc.dram_tensor("wwT", [n_mt, 128, per_mt * KCH * CHUNK], dt_w,
                         kind="ExternalInput")
    out = nc.dram_tensor("out", [npad, ocol], dt_o, kind="ExternalOutput")

    with tile.TileContext(nc) as tc, ExitStack() as ctx:
        xpool = ctx.enter_context(tc.tile_pool(name="x", bufs=1))
        wcpool = ctx.enter_context(tc.tile_pool(name="wc", bufs=1))
        wpool = ctx.enter_context(tc.tile_pool(name="w", bufs=(3 if fp8 else 12)))
        opool = ctx.enter_context(tc.tile_pool(name="o", bufs=8))
        pcp = ctx.enter_context(
            tc.tile_pool(name="pc", bufs=2, space=bass.MemorySpace.PSUM))
        pwp = ctx.enter_context(
            tc.tile_pool(name="pw", bufs=6, space=bass.MemorySpace.PSUM))

        # three independent DMA streams so nothing blocks the W firehose:
        #   sync (SP HWDGE): only the big W chunks, back to back
        #   scalar (ACT HWDGE): wc + per-m-tile x loads
        #   gpsimd (SWDGE): output stores
        wc_sb = wcpool.tile([128, KCH * ncls_p], dt)
        x_sb = xpool.tile([128, n_mt * KCH * 128], dt)
        if fp8:
            # one big x DMA upfront (x is small; W chunks then own the queues)
            nc.scalar.dma_start(x_sb[:], xT[:])
        nc.scalar.dma_start(wc_sb[:], wcT[:])

        wchunk = KCH * gw  # free-dim elems per W DMA (one pw matmul group)
        for m in range(n_mt):
            if not fp8:
                # x columns for this m-tile: [(m*KCH+j)*128 + t]
                nc.scalar.dma_start(x_sb[:, m * KCH * 128:(m + 1) * KCH * 128],
                                    xT[:, m * KCH * 128:(m + 1) * KCH * 128])

            def xcol(j, lo, hi):
                base = (m * KCH + j) * 128
                return x_sb[:, base + lo:base + hi]

            # class logits for these 128 tokens
            pc_ps = pcp.tile([128, ncls_p], F32)
            for j in range(KCH):
                nc.tensor.matmul(
                    pc_ps[:, :],
                    xcol(j, 0, 128),
                    wc_sb[:, j * ncls_p:(j + 1) * ncls_p],
                    start=(j == 0), stop=(j == KCH - 1),
                )

            o_sb = opool.tile([128, ocol], F32)
            nc.vector.tensor_copy(o_sb[:, :NCLS], pc_ps[:, :NCLS])

            if block:
                # word logits: per half, one M=128 matmul of N=gw covering
                # gs classes; only each slot's own class block is kept
                mw_sb = None
                if fp8:
                    # one big W DMA per m-tile (4x fewer, larger transfers)
                    mw_sb = wpool.tile([128, n_half * wchunk], dt_w, tag="w")
                    weng = nc.sync if m % 2 == 0 else nc.scalar
                    weng.dma_start(mw_sb[:], wwT[m])
                for h in range(n_half):
                    if fp8:
                        w_sb, wof = mw_sb, h * wchunk
                    else:
                        w_sb, wof = wpool.tile([128, wchunk], dt, tag="w"), 0
                        weng = (nc.sync if (m * n_half + h) % 2 == 0
                                else nc.scalar)
                        weng.dma_start(
                            w_sb[:], wwT[m][:, h * wchunk:(h + 1) * wchunk])
                    pw_ps = pwp.tile([128, gw], F32, tag="pw")
                    for j in range(KCH):
                        nc.tensor.matmul(
                            pw_ps[:, :],
                            xcol(j, 0, 128),
                            w_sb[:, wof + j * gw:wof + (j + 1) * gw],
                            start=(j == 0), stop=(j == KCH - 1),
                        )
                    if wide:
                        b = gs * C  # 32-row band of this pair
                        nc.vector.tensor_copy(
                            o_sb[h * b:(h + 1) * b, NCLS:],
                            pw_ps[h * b:(h + 1) * b, :])
                    else:
                        for a in range(gs):
                            q = h * gs + a  # slot in m-tile
                            nc.vector.tensor_copy(
                                o_sb[q * C:(q + 1) * C, NCLS:],
                                pw_ps[q * C:(q + 1) * C,
                                      a * CHUNK:(a + 1) * CHUNK])
            else:
                # exact f32: per-slot M=C matmuls col-tiled into one tile
                w_sb = wpool.tile([128, per_mt * KCH * CHUNK], dt, tag="w")
                nc.sync.dma_start(w_sb[:], wwT[m])
                pw_ps = pwp.tile([128, CHUNK], F32, tag="pw")
                for q in range(per_mt):
                    for j in range(KCH):
                        nc.tensor.matmul(
                            pw_ps[q * C:(q + 1) * C, :],
                            xcol(j, q * C, (q + 1) * C),
                            w_sb[:, (q * KCH + j) * CHUNK:
                                 (q * KCH + j + 1) * CHUNK],
                            start=(j == 0), stop=(j == KCH - 1),
                            tile_position=(0, q * C),
                        )
                nc.vector.tensor_copy(o_sb[:, NCLS:], pw_ps[:])

            nc.gpsimd.dma_start(out[m * 128:(m + 1) * 128, :], o_sb[:])

    nc.compile()
    return nc


def _route(cls, mode):
    """Group tokens by class into capacity-padded slots: one slot per class,
    C tokens of capacity.  The (rare) tokens beyond a class's capacity are
    returned as `overflow` and evaluated directly on the host in numpy.

    Returns (C, slots, tok_idx [NCORES, slots*C] int64 token id or -1,
    slot_cls [NCORES, slots] class id per slot, overflow token-id array).
    """
    counts = np.bincount(cls, minlength=NCLS_PAD)
    # coltile (exact f32) needs C to be a multiple of 32 for PSUM col tiling
    cands = ((16, 32, 64, 128) if mode in ("f32r", "bf16", "fp8")
             else (32, 64, 128))
    C = cands[-1]
    for c in cands:
        if int(np.maximum(counts - c, 0).sum()) <= 32:
            C = c
            break

    order = np.argsort(cls, kind="stable")
    starts = np.zeros(NCLS_PAD + 1, np.int64)
    starts[1:] = np.cumsum(counts)

    slots = CPC  # one slot per class owned by the core
    tok_idx = np.full((NCORES, slots * C), -1, np.int64)
    slot_cls = np.full((NCORES, slots), -1, np.int64)
    overflow = []
    for k in range(NCORES):
        for s in range(slots):
            c = k * CPC + s
            lo, cnt = int(starts[c]), int(counts[c])
            n = min(C, cnt)
            slot_cls[k, s] = c
            if n > 0:
                tok_idx[k, s * C:s * C + n] = order[lo:lo + n]
            if cnt > C:
                overflow.append(order[lo + C:lo + cnt])
    overflow = (np.concatenate(overflow) if overflow
                else np.zeros((0,), np.int64))
    return C, slots, tok_idx, slot_cls, overflow


def kernel(x, Wc, bc, Ww, bw, cls_idx, _trace=False, _trace_cores=None,
           _mode=None):
    global LAST_RESULT
    mode = _mode or MODE
    ndt = _NP_DT[mode]
    if ndt is None:
        mode = "f32"
        ndt = np.float32

    x = np.ascontiguousarray(np.asarray(x, np.float32))
    Wc = np.ascontiguousarray(np.asarray(Wc, np.float32))
    bc = np.asarray(bc, np.float32)
    Ww = np.ascontiguousarray(np.asarray(Ww, np.float32))
    bw = np.asarray(bw, np.float32)
    cls = np.asarray(cls_idx).astype(np.int64).ravel()
    N = cls.shape[0]

    C, slots, tok_idx, slot_cls, overflow = _route(cls, mode)
    npad = slots * C
    n_mt = npad // 128
    per_mt = 128 // C
    block = mode in ("f32r", "bf16", "fp8")
    fp8 = mode == "fp8"
    gs = 2 if (block and per_mt >= 2) else 1
    ncls_p = 256 if block else NCLS

    key = (C, slots, mode)
    if key not in _program_cache:
        _program_cache[key] = _build_program(C, slots, mode)
    nc = _program_cache[key]

    # wcT [128, KCH*ncls_p]: wcT[p, j*ncls_p+c] = Wc[c, j*128+p]  (replicated)
    Wc_p = Wc if ncls_p == NCLS else np.concatenate(
        [Wc, np.zeros((ncls_p - NCLS, NHID), np.float32)], 0)
    wcT = np.ascontiguousarray(
        Wc_p.reshape(ncls_p, KCH, 128).transpose(2, 1, 0)
            .reshape(128, KCH * ncls_p).astype(ndt))

    Ww_pad = np.zeros((NCLS_PAD, CHUNK, NHID), np.float32)
    Ww_pad[:NCLS] = Ww

    in_maps = []
    for k in range(NCORES):
        # per-slot k-major weights: tmp[s, j, p, w] = Ww[cls_s, w, j*128+p]
        wsel = Ww_pad[np.maximum(slot_cls[k], 0)]
        wsel[slot_cls[k] < 0] = 0.0
        tmp = wsel.reshape(slots, CHUNK, KCH, 128).transpose(0, 2, 3, 1)
        if gs == 2:
            # group = m-tile (per_mt slots); within: pair r, then j, then
            # the two slots' CHUNK columns side by side
            tmp = tmp.reshape(n_mt, per_mt // 2, 2, KCH, 128, CHUNK)
            tmp = tmp.transpose(0, 4, 1, 3, 2, 5)  # [n_mt,128,pair,j,2,CHUNK]
        else:
            tmp = tmp.reshape(n_mt, per_mt, KCH, 128, CHUNK)
            tmp = tmp.transpose(0, 3, 1, 2, 4)     # [n_mt,128,q,j,CHUNK]
        tmp = tmp.reshape(n_mt, 128, per_mt * KCH * CHUNK)
        if fp8:
            wwT = np.ascontiguousarray(
                (tmp * WSCALE).astype(ml_dtypes.float8_e3m4))
        else:
            wwT = np.ascontiguousarray(tmp.astype(ndt))

        ti = tok_idx[k]
        xk = x[np.maximum(ti, 0)]
        xk[ti < 0] = 0.0
        # xT[p, (m*KCH+j)*128 + t] = xk[m*128+t, j*128+p]
        xT = np.ascontiguousarray(
            xk.reshape(n_mt, 128, KCH, 128).transpose(3, 0, 2, 1)
              .reshape(128, n_mt * KCH * 128).astype(ndt))
        in_maps.append({"xT": xT, "wcT": wcT, "wwT": wwT})

    LAST_RESULT = run_bass_kernel_spmd(
        nc, in_maps, list(range(NCORES)), trace=_trace,
        trace_cores=(_trace_cores if _trace else None))

    wide = block and C == 16
    out = np.zeros((N, NCOL), np.float32)
    if wide:
        # row r of a core's output holds its pair's full 2*CHUNK block;
        # slot parity selects which CHUNK half is this row's class
        a_row = (np.arange(npad) // C) % 2
    for k in range(NCORES):
        ok = np.asarray(LAST_RESULT.results[k]["out"]).astype(np.float32)
        if fp8:
            ok[:, NCLS:] *= 1.0 / WSCALE   # undo host prescale of Ww
        if wide:
            words = np.where((a_row == 0)[:, None],
                             ok[:, NCLS:NCLS + CHUNK],
                             ok[:, NCLS + CHUNK:NCLS + 2 * CHUNK])
            ok = np.concatenate([ok[:, :NCLS], words], 1)
        valid = tok_idx[k] >= 0
        out[tok_idx[k][valid]] = ok[valid]

    if overflow.size:
        # rare capacity-overflow tokens: evaluate directly on the host
        xo = x[overflow]                                   # [no, NHID]
        out[overflow, :NCLS] = xo @ Wc.T
        co = cls[overflow]
        out[overflow, NCLS:] = np.einsum(
            "nkh,nh->nk", Ww[co], xo, optimize=True)

    out[:, :NCLS] += bc
    out[:, NCLS:] += bw[cls]
    return out



# revision 14
# speedup vs baseline: 1.2457x; 1.1848x over previous
"""Class-based decoder (MoE-style routing) on 8 trn2 NeuronCores.

Strategy: expert-parallel. Classes are padded 250->256 and split 32 per core.
On the host, tokens are grouped by class into capacity-padded slots (C tokens
per class slot, C in {32,64,128}); class slots that overflow C spill into
extra slots holding a duplicate of the class weights.  Each core receives:
  - xT   [128, n_mt*KCH*128]   its padded tokens, pre-transposed k-major
  - wcT  [128, KCH*NCLS_P]     the (replicated) class-decoder weights, k-major
  - wwT  [n_grp, 128, GRP*KCH*CHUNK]  its word-decoder shard, k-major, grouped
and computes, for every 128-token tile, the class logits (x @ Wc.T) and the
per-class word logits (x_c @ Ww[c].T) as PE matmuls accumulating K=512 over
4 PSUM chunks.  Class slots of a tile are col-tiled into one PSUM tile.
Biases (zero in practice, but handled for correctness) are added on the host
during the final unpermute.

Precision modes:
  f32  : exact fp32 matmuls (2-pass LOW/HIGH on PE; bit-exact, slowest)
  f32r : fp32 data, single-pass PE (TF32-like rounding). Classes are paired
         into N=400 matmuls and NCLS padded to 256 so the free dim is >=256,
         where f32r streams at full rate.
  bf16 : weights/activations cast to bf16 (halves the big W DMA)
  fp8  : word-decoder weights in float8e3 (E3M4), prescaled by 128 so the
         uniform(-0.1,0.1) values land in the normal range; x/Wc stay bf16
         (PE allows mixed non-fp32 operand dtypes) and the output is stored
         bf16.  Halves the dominant W DMA again vs bf16; ~0.9% rel err.
"""

import numpy as np
from contextlib import ExitStack

import concourse.bass as bass
import concourse.bacc as bacc
import concourse.tile as tile
import concourse.mybir as mybir
from concourse.bass_utils import run_bass_kernel_spmd

NHID = 512
NCLS = 250
CHUNK = 200
NCORES = 8
KCH = NHID // 128          # 4 contraction chunks of 128
NCLS_PAD = 256             # classes padded so each core owns an equal shard
CPC = NCLS_PAD // NCORES   # classes per core
NCOL = NCLS + CHUNK        # 450 output columns
F32 = mybir.dt.float32

MODE = "fp8"               # default precision mode; see module docstring
WSCALE = 128.0             # fp8 mode: host prescale of Ww (undone after run)

LAST_RESULT = None         # BassKernelResults of the most recent device run
_program_cache = {}

_MM_DT = {"f32": mybir.dt.float32, "f32r": mybir.dt.float32r,
          "bf16": mybir.dt.bfloat16, "fp8": mybir.dt.bfloat16}
_NP_DT = {"f32": np.float32, "f32r": np.float32, "bf16": None, "fp8": None}
try:
    import ml_dtypes
    _NP_DT["bf16"] = ml_dtypes.bfloat16
    _NP_DT["fp8"] = ml_dtypes.bfloat16   # x/Wc dtype; Ww uses float8_e3m4
except ImportError:
    pass


def _build_program_fp8(C, slots):
    """fp8 SPMD program, restructured for latency:
      - x on sync + wc on scalar first, then all W m-tile chunks (alternating
        queues, fully resident in SBUF: bufs=n_mt) -- the DMA streams never
        stall on compute.
      - a dummy-matmul warmup chain occupies the PE from kernel start so the
        HAM clock gate reaches 8/8 (~3.4us) before the real matmuls arrive.
      - all class matmuls run first (they only need x+wc), then the word
        matmuls chase the W stream m-tile by m-tile.
      - PSUM->SBUF copies are split between DVE and ACT so neither paces the
        word-matmul stream.
    """
    n_mt = (slots * C) // 128
    npad = slots * C
    per_mt = 128 // C
    gs = 2 if per_mt >= 2 else 1
    gw = gs * CHUNK
    n_half = per_mt // gs
    ncls_p = 256
    wide = C == 16
    ocol = NCLS + (gw if wide else CHUNK)
    BF16 = mybir.dt.bfloat16
    FP8 = mybir.dt.float8e3
    wchunk = KCH * gw

    nc = bacc.Bacc("TRN2", target_bir_lowering=False, debug=False,
                   num_devices=NCORES)
    xT = nc.dram_tensor("xT", [128, n_mt * KCH * 128], BF16,
                        kind="ExternalInput")
    wcT = nc.dram_tensor("wcT", [128, KCH * ncls_p], BF16,
                         kind="ExternalInput")
    wwT = nc.dram_tensor("wwT", [n_mt, 128, per_mt * KCH * CHUNK], FP8,
                         kind="ExternalInput")
    out = nc.dram_tensor("out", [npad, ocol], BF16, kind="ExternalOutput")

    with tile.TileContext(nc) as tc, ExitStack() as ctx:
        xpool = ctx.enter_context(tc.tile_pool(name="x", bufs=1))
        wcpool = ctx.enter_context(tc.tile_pool(name="wc", bufs=1))
        wpool = ctx.enter_context(tc.tile_pool(name="w", bufs=n_mt))
        opool = ctx.enter_context(tc.tile_pool(name="o", bufs=n_mt))
        wupool = ctx.enter_context(tc.tile_pool(name="wu", bufs=1))
        pcp = ctx.enter_context(
            tc.tile_pool(name="pc", bufs=2, space=bass.MemorySpace.PSUM))
        pwp = ctx.enter_context(
            tc.tile_pool(name="pw", bufs=5, space=bass.MemorySpace.PSUM))

        # ---- all input DMAs, issued upfront on the two HWDGE queues ----
        x_sb = xpool.tile([128, n_mt * KCH * 128], BF16)
        nc.sync.dma_start(x_sb[:], xT[:])
        wc_sb = wcpool.tile([128, KCH * ncls_p], BF16)
        nc.scalar.dma_start(wc_sb[:], wcT[:])
        w_sbs = []
        for m in range(n_mt):
            w_sb = wpool.tile([128, per_mt * KCH * CHUNK], FP8, tag="w")
            (nc.sync if m % 2 == 0 else nc.scalar).dma_start(w_sb[:], wwT[m])
            w_sbs.append(w_sb)

        # ---- PE warmup: ~4.5us of dummy matmuls so HAM hits 8/8 early ----
        wu_sb = wupool.tile([128, 384], BF16)
        nc.gpsimd.memset(wu_sb[:], 0)
        wu_ps = pwp.tile([128, gw], F32, tag="pw")
        for i in range(12):
            nc.tensor.matmul(wu_ps[:, :384], wu_sb[:, :128], wu_sb[:, :384],
                             start=True, stop=True)

        def xcol(m, j):
            base = (m * KCH + j) * 128
            return x_sb[:, base:base + 128]

        # ---- class logits for every m-tile first (only need x + wc) ----
        o_sbs = []
        for m in range(n_mt):
            pc_ps = pcp.tile([128, ncls_p], F32)
            for j in range(KCH):
                nc.tensor.matmul(
                    pc_ps[:, :], xcol(m, j),
                    wc_sb[:, j * ncls_p:(j + 1) * ncls_p],
                    start=(j == 0), stop=(j == KCH - 1))
            o_sb = opool.tile([128, ocol], BF16)
            nc.scalar.copy(o_sb[:, :NCLS], pc_ps[:, :NCLS])
            o_sbs.append(o_sb)

        # ---- word logits, m-tile by m-tile as the W stream lands ----
        for m in range(n_mt):
            o_sb = o_sbs[m]
            for h in range(n_half):
                pw_ps = pwp.tile([128, gw], F32, tag="pw")
                for j in range(KCH):
                    nc.tensor.matmul(
                        pw_ps[:, :], xcol(m, j),
                        w_sbs[m][:, h * wchunk + j * gw:
                                 h * wchunk + (j + 1) * gw],
                        start=(j == 0), stop=(j == KCH - 1))
                ceng = nc.vector.tensor_copy if h % 2 == 0 else nc.scalar.copy
                if wide:
                    b = gs * C
                    ceng(o_sb[h * b:(h + 1) * b, NCLS:],
                         pw_ps[h * b:(h + 1) * b, :])
                else:
                    for a in range(gs):
                        q = h * gs + a
                        ceng(o_sb[q * C:(q + 1) * C, NCLS:],
                             pw_ps[q * C:(q + 1) * C,
                                   a * CHUNK:(a + 1) * CHUNK])
            nc.gpsimd.dma_start(out[m * 128:(m + 1) * 128, :], o_sb[:])

    nc.compile()
    return nc


def _build_program(C, slots, mode):
    """One SPMD program: slots class-slots of C tokens each, per core.

    f32 uses the "coltile" scheme: per class slot, an M=C matmul col-tiled
    into a shared PSUM tile (exact 2-pass fp32).
    f32r/bf16 use the "block" scheme: every matmul is M=128 (all slots of an
    m-tile), and the word logits come as per_mt//2 halves of N=2*CHUNK whose
    off-diagonal class blocks are discarded by the PSUM->SBUF copies.  This
    keeps N>=256 (full-rate f32r) and NumWeights=128 (FWL weight loads).
    """
    n_mt = (slots * C) // 128  # 128-token m-tiles
    npad = slots * C
    per_mt = 128 // C          # class slots per m-tile
    block = mode in ("f32r", "bf16", "fp8")
    fp8 = mode == "fp8"
    # class slots per pw matmul and word-columns per pw matmul
    gs = 2 if (block and per_mt >= 2) else 1
    gw = gs * CHUNK            # 400 paired / 200 single
    n_half = per_mt // gs      # pw matmul groups per m-tile
    ncls_p = 256 if block else NCLS  # N>=256 keeps f32r at full rate
    # C=16 diag copies would need 16-partition bases (illegal); store each
    # 32-row band's full pair block instead and let the host pick the diagonal
    wide = block and C == 16
    ocol = NCLS + (gw if wide else CHUNK)
    dt = _MM_DT[mode]
    dt_w = mybir.dt.float8e3 if fp8 else dt
    dt_o = mybir.dt.bfloat16 if fp8 else F32

    nc = bacc.Bacc("TRN2", target_bir_lowering=False, debug=False,
                   num_devices=NCORES)
    xT = nc.dram_tensor("xT", [128, n_mt * KCH * 128], dt, kind="ExternalInput")
    wcT = nc.dram_tensor("wcT", [128, KCH * ncls_p], dt, kind="ExternalInput")
    # W groups: one DMA per m-tile worth of class slots
    wwT = nc.dram_tensor("wwT", [n_mt, 128, per_mt * KCH * CHUNK], dt_w,
                         kind="ExternalInput")
    out = nc.dram_tensor("out", [npad, ocol], dt_o, kind="ExternalOutput")

    with tile.TileContext(nc) as tc, ExitStack() as ctx:
        xpool = ctx.enter_context(tc.tile_pool(name="x", bufs=1))
        wcpool = ctx.enter_context(tc.tile_pool(name="wc", bufs=1))
        wpool = ctx.enter_context(tc.tile_pool(name="w", bufs=(3 if fp8 else 12)))
        opool = ctx.enter_context(tc.tile_pool(name="o", bufs=8))
        pcp = ctx.enter_context(
            tc.tile_pool(name="pc", bufs=2, space=bass.MemorySpace.PSUM))
        pwp = ctx.enter_context(
            tc.tile_pool(name="pw", bufs=6, space=bass.MemorySpace.PSUM))

        # three independent DMA streams so nothing blocks the W firehose:
        #   sync (SP HWDGE): only the big W chunks, back to back
        #   scalar (ACT HWDGE): wc + per-m-tile x loads
        #   gpsimd (SWDGE): output stores
        wc_sb = wcpool.tile([128, KCH * ncls_p], dt)
        x_sb = xpool.tile([128, n_mt * KCH * 128], dt)
        if fp8:
            # one big x DMA upfront (x is small; W chunks then own the queues)
            nc.scalar.dma_start(x_sb[:], xT[:])
        nc.scalar.dma_start(wc_sb[:], wcT[:])

        wchunk = KCH * gw  # free-dim elems per W DMA (one pw matmul group)
        for m in range(n_mt):
            if not fp8:
                # x columns for this m-tile: [(m*KCH+j)*128 + t]
                nc.scalar.dma_start(x_sb[:, m * KCH * 128:(m + 1) * KCH * 128],
                                    xT[:, m * KCH * 128:(m + 1) * KCH * 128])

            def xcol(j, lo, hi):
                base = (m * KCH + j) * 128
                return x_sb[:, base + lo:base + hi]

            # class logits for these 128 tokens
            pc_ps = pcp.tile([128, ncls_p], F32)
            for j in range(KCH):
                nc.tensor.matmul(
                    pc_ps[:, :],
                    xcol(j, 0, 128),
                    wc_sb[:, j * ncls_p:(j + 1) * ncls_p],
                    start=(j == 0), stop=(j == KCH - 1),
                )

            o_sb = opool.tile([128, ocol], F32)
            nc.vector.tensor_copy(o_sb[:, :NCLS], pc_ps[:, :NCLS])

            if block:
                # word logits: per half, one M=128 matmul of N=gw covering
                # gs classes; only each slot's own class block is kept
                mw_sb = None
                if fp8:
                    # one big W DMA per m-tile (4x fewer, larger transfers)
                    mw_sb = wpool.tile([128, n_half * wchunk], dt_w, tag="w")
                    weng = nc.sync if m % 2 == 0 else nc.scalar
                    weng.dma_start(mw_sb[:], wwT[m])
                for h in range(n_half):
                    if fp8:
                        w_sb, wof = mw_sb, h * wchunk
                    else:
                        w_sb, wof = wpool.tile([128, wchunk], dt, tag="w"), 0
                        weng = (nc.sync if (m * n_half + h) % 2 == 0
                                else nc.scalar)
                        weng.dma_start(
                            w_sb[:], wwT[m][:, h * wchunk:(h + 1) * wchunk])
                    pw_ps = pwp.tile([128, gw], F32, tag="pw")
                    for j in range(KCH):
                        nc.tensor.matmul(
                            pw_ps[:, :],
                            xcol(j, 0, 128),
                            w_sb[:, wof + j * gw:wof + (j + 1) * gw],
                            start=(j == 0), stop=(j == KCH - 1),
                        )
                    if wide:
                        b = gs * C  # 32-row band of this pair
                        nc.vector.tensor_copy(
                            o_sb[h * b:(h + 1) * b, NCLS:],
                            pw_ps[h * b:(h + 1) * b, :])
                    else:
                        for a in range(gs):
                            q = h * gs + a  # slot in m-tile
                            nc.vector.tensor_copy(
                                o_sb[q * C:(q + 1) * C, NCLS:],
                                pw_ps[q * C:(q + 1) * C,
                                      a * CHUNK:(a + 1) * CHUNK])
            else:
                # exact f32: per-slot M=C matmuls col-tiled into one tile
                w_sb = wpool.tile([128, per_mt * KCH * CHUNK], dt, tag="w")
                nc.sync.dma_start(w_sb[:], wwT[m])
                pw_ps = pwp.tile([128, CHUNK], F32, tag="pw")
                for q in range(per_mt):
                    for j in range(KCH):
                        nc.tensor.matmul(
                            pw_ps[q * C:(q + 1) * C, :],
                            xcol(j, q * C, (q + 1) * C),
                            w_sb[:, (q * KCH + j) * CHUNK:
                                 (q * KCH + j + 1) * CHUNK],
                            start=(j == 0), stop=(j == KCH - 1),
                            tile_position=(0, q * C),
                        )
                nc.vector.tensor_copy(o_sb[:, NCLS:], pw_ps[:])

            nc.gpsimd.dma_start(out[m * 128:(m + 1) * 128, :], o_sb[:])

    nc.compile()
    return nc


def _route(cls, mode):
    """Group tokens by class into capacity-padded slots: one slot per class,
    C tokens of capacity.  The (rare) tokens beyond a class's capacity are
    returned as `overflow` and evaluated directly on the host in numpy.

    Returns (C, slots, tok_idx [NCORES, slots*C] int64 token id or -1,
    slot_cls [NCORES, slots] class id per slot, overflow token-id array).
    """
    counts = np.bincount(cls, minlength=NCLS_PAD)
    # coltile (exact f32) needs C to be a multiple of 32 for PSUM col tiling
    cands = ((16, 32, 64, 128) if mode in ("f32r", "bf16", "fp8")
             else (32, 64, 128))
    C = cands[-1]
    for c in cands:
        if int(np.maximum(counts - c, 0).sum()) <= 32:
            C = c
            break

    order = np.argsort(cls, kind="stable")
    starts = np.zeros(NCLS_PAD + 1, np.int64)
    starts[1:] = np.cumsum(counts)

    slots = CPC  # one slot per class owned by the core
    tok_idx = np.full((NCORES, slots * C), -1, np.int64)
    slot_cls = np.full((NCORES, slots), -1, np.int64)
    overflow = []
    for k in range(NCORES):
        for s in range(slots):
            c = k * CPC + s
            lo, cnt = int(starts[c]), int(counts[c])
            n = min(C, cnt)
            slot_cls[k, s] = c
            if n > 0:
                tok_idx[k, s * C:s * C + n] = order[lo:lo + n]
            if cnt > C:
                overflow.append(order[lo + C:lo + cnt])
    overflow = (np.concatenate(overflow) if overflow
                else np.zeros((0,), np.int64))
    return C, slots, tok_idx, slot_cls, overflow


def kernel(x, Wc, bc, Ww, bw, cls_idx, _trace=False, _trace_cores=None,
           _mode=None):
    global LAST_RESULT
    mode = _mode or MODE
    ndt = _NP_DT[mode]
    if ndt is None:
        mode = "f32"
        ndt = np.float32

    x = np.ascontiguousarray(np.asarray(x, np.float32))
    Wc = np.ascontiguousarray(np.asarray(Wc, np.float32))
    bc = np.asarray(bc, np.float32)
    Ww = np.ascontiguousarray(np.asarray(Ww, np.float32))
    bw = np.asarray(bw, np.float32)
    cls = np.asarray(cls_idx).astype(np.int64).ravel()
    N = cls.shape[0]

    C, slots, tok_idx, slot_cls, overflow = _route(cls, mode)
    npad = slots * C
    n_mt = npad // 128
    per_mt = 128 // C
    block = mode in ("f32r", "bf16", "fp8")
    fp8 = mode == "fp8"
    gs = 2 if (block and per_mt >= 2) else 1
    ncls_p = 256 if block else NCLS

    key = (C, slots, mode)
    if key not in _program_cache:
        _program_cache[key] = (_build_program_fp8(C, slots) if fp8
                               else _build_program(C, slots, mode))
    nc = _program_cache[key]

    # wcT [128, KCH*ncls_p]: wcT[p, j*ncls_p+c] = Wc[c, j*128+p]  (replicated)
    Wc_p = Wc if ncls_p == NCLS else np.concatenate(
        [Wc, np.zeros((ncls_p - NCLS, NHID), np.float32)], 0)
    wcT = np.ascontiguousarray(
        Wc_p.reshape(ncls_p, KCH, 128).transpose(2, 1, 0)
            .reshape(128, KCH * ncls_p).astype(ndt))

    Ww_pad = np.zeros((NCLS_PAD, CHUNK, NHID), np.float32)
    Ww_pad[:NCLS] = Ww

    in_maps = []
    for k in range(NCORES):
        # per-slot k-major weights: tmp[s, j, p, w] = Ww[cls_s, w, j*128+p]
        wsel = Ww_pad[np.maximum(slot_cls[k], 0)]
        wsel[slot_cls[k] < 0] = 0.0
        tmp = wsel.reshape(slots, CHUNK, KCH, 128).transpose(0, 2, 3, 1)
        if gs == 2:
            # group = m-tile (per_mt slots); within: pair r, then j, then
            # the two slots' CHUNK columns side by side
            tmp = tmp.reshape(n_mt, per_mt // 2, 2, KCH, 128, CHUNK)
            tmp = tmp.transpose(0, 4, 1, 3, 2, 5)  # [n_mt,128,pair,j,2,CHUNK]
        else:
            tmp = tmp.reshape(n_mt, per_mt, KCH, 128, CHUNK)
            tmp = tmp.transpose(0, 3, 1, 2, 4)     # [n_mt,128,q,j,CHUNK]
        tmp = tmp.reshape(n_mt, 128, per_mt * KCH * CHUNK)
        if fp8:
            wwT = np.ascontiguousarray(
                (tmp * WSCALE).astype(ml_dtypes.float8_e3m4))
        else:
            wwT = np.ascontiguousarray(tmp.astype(ndt))

        ti = tok_idx[k]
        xk = x[np.maximum(ti, 0)]
        xk[ti < 0] = 0.0
        # xT[p, (m*KCH+j)*128 + t] = xk[m*128+t, j*128+p]
        xT = np.ascontiguousarray(
            xk.reshape(n_mt, 128, KCH, 128).transpose(3, 0, 2, 1)
              .reshape(128, n_mt * KCH * 128).astype(ndt))
        in_maps.append({"xT": xT, "wcT": wcT, "wwT": wwT})

    LAST_RESULT = run_bass_kernel_spmd(
        nc, in_maps, list(range(NCORES)), trace=_trace,
        trace_cores=(_trace_cores if _trace else None))

    wide = block and C == 16
    out = np.zeros((N, NCOL), np.float32)
    if wide:
        # row r of a core's output holds its pair's full 2*CHUNK block;
        # slot parity selects which CHUNK half is this row's class
        a_row = (np.arange(npad) // C) % 2
    for k in range(NCORES):
        ok = np.asarray(LAST_RESULT.results[k]["out"]).astype(np.float32)
        if fp8:
            ok[:, NCLS:] *= 1.0 / WSCALE   # undo host prescale of Ww
        if wide:
            words = np.where((a_row == 0)[:, None],
                             ok[:, NCLS:NCLS + CHUNK],
                             ok[:, NCLS + CHUNK:NCLS + 2 * CHUNK])
            ok = np.concatenate([ok[:, :NCLS], words], 1)
        valid = tok_idx[k] >= 0
        out[tok_idx[k][valid]] = ok[valid]

    if overflow.size:
        # rare capacity-overflow tokens: evaluate directly on the host
        xo = x[overflow]                                   # [no, NHID]
        out[overflow, :NCLS] = xo @ Wc.T
        co = cls[overflow]
        out[overflow, NCLS:] = np.einsum(
            "nkh,nh->nk", Ww[co], xo, optimize=True)

    out[:, :NCLS] += bc
    out[:, NCLS:] += bw[cls]
    return out



# revision 15
# speedup vs baseline: 1.2689x; 1.0187x over previous
"""Class-based decoder (MoE-style routing) on 8 trn2 NeuronCores.

Strategy: expert-parallel. Classes are padded 250->256 and split 32 per core.
On the host, tokens are grouped by class into capacity-padded slots (C tokens
per class slot, C in {32,64,128}); class slots that overflow C spill into
extra slots holding a duplicate of the class weights.  Each core receives:
  - xT   [128, n_mt*KCH*128]   its padded tokens, pre-transposed k-major
  - wcT  [128, KCH*NCLS_P]     the (replicated) class-decoder weights, k-major
  - wwT  [n_grp, 128, GRP*KCH*CHUNK]  its word-decoder shard, k-major, grouped
and computes, for every 128-token tile, the class logits (x @ Wc.T) and the
per-class word logits (x_c @ Ww[c].T) as PE matmuls accumulating K=512 over
4 PSUM chunks.  Class slots of a tile are col-tiled into one PSUM tile.
Biases (zero in practice, but handled for correctness) are added on the host
during the final unpermute.

Precision modes:
  f32  : exact fp32 matmuls (2-pass LOW/HIGH on PE; bit-exact, slowest)
  f32r : fp32 data, single-pass PE (TF32-like rounding). Classes are paired
         into N=400 matmuls and NCLS padded to 256 so the free dim is >=256,
         where f32r streams at full rate.
  bf16 : weights/activations cast to bf16 (halves the big W DMA)
  fp8  : word-decoder weights in float8e3 (E3M4), prescaled by 128 so the
         uniform(-0.1,0.1) values land in the normal range; x/Wc stay bf16
         (PE allows mixed non-fp32 operand dtypes) and the output is stored
         bf16.  Halves the dominant W DMA again vs bf16; ~0.9% rel err.
"""

import numpy as np
from contextlib import ExitStack

import concourse.bass as bass
import concourse.bacc as bacc
import concourse.tile as tile
import concourse.mybir as mybir
from concourse.bass_utils import run_bass_kernel_spmd

NHID = 512
NCLS = 250
CHUNK = 200
NCORES = 8
KCH = NHID // 128          # 4 contraction chunks of 128
NCLS_PAD = 256             # classes padded so each core owns an equal shard
CPC = NCLS_PAD // NCORES   # classes per core
NCOL = NCLS + CHUNK        # 450 output columns
F32 = mybir.dt.float32

MODE = "fp8"               # default precision mode; see module docstring
WSCALE = 128.0             # fp8 mode: host prescale of Ww (undone after run)

LAST_RESULT = None         # BassKernelResults of the most recent device run
_program_cache = {}

_MM_DT = {"f32": mybir.dt.float32, "f32r": mybir.dt.float32r,
          "bf16": mybir.dt.bfloat16, "fp8": mybir.dt.bfloat16}
_NP_DT = {"f32": np.float32, "f32r": np.float32, "bf16": None, "fp8": None}
try:
    import ml_dtypes
    _NP_DT["bf16"] = ml_dtypes.bfloat16
    _NP_DT["fp8"] = ml_dtypes.bfloat16   # x/Wc dtype; Ww uses float8_e3m4
except ImportError:
    pass


def _build_program_fp8(C, slots):
    """fp8 SPMD program, restructured for latency:
      - x on sync + wc on scalar first, then all W m-tile chunks (alternating
        queues, fully resident in SBUF: bufs=n_mt) -- the DMA streams never
        stall on compute.
      - a dummy-matmul warmup chain occupies the PE from kernel start so the
        HAM clock gate reaches 8/8 (~3.4us) before the real matmuls arrive.
      - all class matmuls run first (they only need x+wc), then the word
        matmuls chase the W stream m-tile by m-tile.
      - PSUM->SBUF copies are split between DVE and ACT so neither paces the
        word-matmul stream.
    """
    n_mt = (slots * C) // 128
    npad = slots * C
    per_mt = 128 // C
    gs = 2 if per_mt >= 2 else 1
    gw = gs * CHUNK
    n_half = per_mt // gs
    ncls_p = 256
    wide = C == 16
    ocol = NCLS + (gw if wide else CHUNK)
    BF16 = mybir.dt.bfloat16
    FP8 = mybir.dt.float8e3
    wchunk = KCH * gw

    nc = bacc.Bacc("TRN2", target_bir_lowering=False, debug=False,
                   num_devices=NCORES)
    xT = nc.dram_tensor("xT", [128, n_mt * KCH * 128], BF16,
                        kind="ExternalInput")
    wcT = nc.dram_tensor("wcT", [128, KCH * ncls_p], BF16,
                         kind="ExternalInput")
    wwT = nc.dram_tensor("wwT", [n_mt, 128, per_mt * KCH * CHUNK], FP8,
                         kind="ExternalInput")
    out = nc.dram_tensor("out", [npad, ocol], BF16, kind="ExternalOutput")

    hchunk = (n_half // 2) * wchunk  # free-dim elems per half-m-tile W DMA

    with tile.TileContext(nc) as tc, ExitStack() as ctx:
        xpool = ctx.enter_context(tc.tile_pool(name="x", bufs=1))
        wcpool = ctx.enter_context(tc.tile_pool(name="wc", bufs=1))
        wpool = ctx.enter_context(tc.tile_pool(name="w", bufs=2 * n_mt))
        opool = ctx.enter_context(tc.tile_pool(name="o", bufs=n_mt))
        wupool = ctx.enter_context(tc.tile_pool(name="wu", bufs=1))
        pcp = ctx.enter_context(
            tc.tile_pool(name="pc", bufs=2, space=bass.MemorySpace.PSUM))
        pwp = ctx.enter_context(
            tc.tile_pool(name="pw", bufs=5, space=bass.MemorySpace.PSUM))

        # ---- all input DMAs upfront, spread over the two HWDGE queues.
        # W comes as half-m-tile chunks; the first words chunk leads the
        # scalar queue (x leads sync) so words m0 can start ~2us earlier.
        x_sb = xpool.tile([128, n_mt * KCH * 128], BF16)
        wc_sb = wcpool.tile([128, KCH * ncls_p], BF16)
        w_sbs = [[None, None] for _ in range(n_mt)]

        def wdma(eng, m, half):
            w_sb = wpool.tile([128, hchunk], FP8, tag="w")
            eng.dma_start(w_sb[:],
                          wwT[m][:, half * hchunk:(half + 1) * hchunk])
            w_sbs[m][half] = w_sb

        nc.sync.dma_start(x_sb[:], xT[:])
        wdma(nc.scalar, 0, 0)
        nc.scalar.dma_start(wc_sb[:], wcT[:])
        wdma(nc.sync, 0, 1)
        for m in range(1, n_mt):
            wdma(nc.sync, m, 0)
            wdma(nc.scalar, m, 1)

        # ---- PE warmup: dummy matmuls so HAM hits 8/8 before real work ----
        wu_sb = wupool.tile([128, 384], BF16)
        nc.vector.memset(wu_sb[:], 0)
        wu_ps = pwp.tile([128, gw], F32, tag="pw")
        for i in range(14):
            nc.tensor.matmul(wu_ps[:, :384], wu_sb[:, :128], wu_sb[:, :384],
                             start=True, stop=True)

        def xcol(m, j):
            base = (m * KCH + j) * 128
            return x_sb[:, base:base + 128]

        # ---- per m-tile: class logits (x+wc only), then words as W lands --
        for m in range(n_mt):
            pc_ps = pcp.tile([128, ncls_p], F32)
            for j in range(KCH):
                nc.tensor.matmul(
                    pc_ps[:, :], xcol(m, j),
                    wc_sb[:, j * ncls_p:(j + 1) * ncls_p],
                    start=(j == 0), stop=(j == KCH - 1))
            o_sb = opool.tile([128, ocol], BF16)
            nc.scalar.copy(o_sb[:, :NCLS], pc_ps[:, :NCLS])

            for h in range(n_half):
                w_sb = w_sbs[m][h // (n_half // 2)]
                wof = (h % (n_half // 2)) * wchunk
                pw_ps = pwp.tile([128, gw], F32, tag="pw")
                for j in range(KCH):
                    nc.tensor.matmul(
                        pw_ps[:, :], xcol(m, j),
                        w_sb[:, wof + j * gw:wof + (j + 1) * gw],
                        start=(j == 0), stop=(j == KCH - 1))
                ceng = nc.vector.tensor_copy if h % 2 == 0 else nc.scalar.copy
                if wide:
                    b = gs * C
                    ceng(o_sb[h * b:(h + 1) * b, NCLS:],
                         pw_ps[h * b:(h + 1) * b, :])
                else:
                    for a in range(gs):
                        q = h * gs + a
                        ceng(o_sb[q * C:(q + 1) * C, NCLS:],
                             pw_ps[q * C:(q + 1) * C,
                                   a * CHUNK:(a + 1) * CHUNK])
            nc.gpsimd.dma_start(out[m * 128:(m + 1) * 128, :], o_sb[:])

    nc.compile()
    return nc


def _build_program(C, slots, mode):
    """One SPMD program: slots class-slots of C tokens each, per core.

    f32 uses the "coltile" scheme: per class slot, an M=C matmul col-tiled
    into a shared PSUM tile (exact 2-pass fp32).
    f32r/bf16 use the "block" scheme: every matmul is M=128 (all slots of an
    m-tile), and the word logits come as per_mt//2 halves of N=2*CHUNK whose
    off-diagonal class blocks are discarded by the PSUM->SBUF copies.  This
    keeps N>=256 (full-rate f32r) and NumWeights=128 (FWL weight loads).
    """
    n_mt = (slots * C) // 128  # 128-token m-tiles
    npad = slots * C
    per_mt = 128 // C          # class slots per m-tile
    block = mode in ("f32r", "bf16", "fp8")
    fp8 = mode == "fp8"
    # class slots per pw matmul and word-columns per pw matmul
    gs = 2 if (block and per_mt >= 2) else 1
    gw = gs * CHUNK            # 400 paired / 200 single
    n_half = per_mt // gs      # pw matmul groups per m-tile
    ncls_p = 256 if block else NCLS  # N>=256 keeps f32r at full rate
    # C=16 diag copies would need 16-partition bases (illegal); store each
    # 32-row band's full pair block instead and let the host pick the diagonal
    wide = block and C == 16
    ocol = NCLS + (gw if wide else CHUNK)
    dt = _MM_DT[mode]
    dt_w = mybir.dt.float8e3 if fp8 else dt
    dt_o = mybir.dt.bfloat16 if fp8 else F32

    nc = bacc.Bacc("TRN2", target_bir_lowering=False, debug=False,
                   num_devices=NCORES)
    xT = nc.dram_tensor("xT", [128, n_mt * KCH * 128], dt, kind="ExternalInput")
    wcT = nc.dram_tensor("wcT", [128, KCH * ncls_p], dt, kind="ExternalInput")
    # W groups: one DMA per m-tile worth of class slots
    wwT = nc.dram_tensor("wwT", [n_mt, 128, per_mt * KCH * CHUNK], dt_w,
                         kind="ExternalInput")
    out = nc.dram_tensor("out", [npad, ocol], dt_o, kind="ExternalOutput")

    with tile.TileContext(nc) as tc, ExitStack() as ctx:
        xpool = ctx.enter_context(tc.tile_pool(name="x", bufs=1))
        wcpool = ctx.enter_context(tc.tile_pool(name="wc", bufs=1))
        wpool = ctx.enter_context(tc.tile_pool(name="w", bufs=(3 if fp8 else 12)))
        opool = ctx.enter_context(tc.tile_pool(name="o", bufs=8))
        pcp = ctx.enter_context(
            tc.tile_pool(name="pc", bufs=2, space=bass.MemorySpace.PSUM))
        pwp = ctx.enter_context(
            tc.tile_pool(name="pw", bufs=6, space=bass.MemorySpace.PSUM))

        # three independent DMA streams so nothing blocks the W firehose:
        #   sync (SP HWDGE): only the big W chunks, back to back
        #   scalar (ACT HWDGE): wc + per-m-tile x loads
        #   gpsimd (SWDGE): output stores
        wc_sb = wcpool.tile([128, KCH * ncls_p], dt)
        x_sb = xpool.tile([128, n_mt * KCH * 128], dt)
        if fp8:
            # one big x DMA upfront (x is small; W chunks then own the queues)
            nc.scalar.dma_start(x_sb[:], xT[:])
        nc.scalar.dma_start(wc_sb[:], wcT[:])

        wchunk = KCH * gw  # free-dim elems per W DMA (one pw matmul group)
        for m in range(n_mt):
            if not fp8:
                # x columns for this m-tile: [(m*KCH+j)*128 + t]
                nc.scalar.dma_start(x_sb[:, m * KCH * 128:(m + 1) * KCH * 128],
                                    xT[:, m * KCH * 128:(m + 1) * KCH * 128])

            def xcol(j, lo, hi):
                base = (m * KCH + j) * 128
                return x_sb[:, base + lo:base + hi]

            # class logits for these 128 tokens
            pc_ps = pcp.tile([128, ncls_p], F32)
            for j in range(KCH):
                nc.tensor.matmul(
                    pc_ps[:, :],
                    xcol(j, 0, 128),
                    wc_sb[:, j * ncls_p:(j + 1) * ncls_p],
                    start=(j == 0), stop=(j == KCH - 1),
                )

            o_sb = opool.tile([128, ocol], F32)
            nc.vector.tensor_copy(o_sb[:, :NCLS], pc_ps[:, :NCLS])

            if block:
                # word logits: per half, one M=128 matmul of N=gw covering
                # gs classes; only each slot's own class block is kept
                mw_sb = None
                if fp8:
                    # one big W DMA per m-tile (4x fewer, larger transfers)
                    mw_sb = wpool.tile([128, n_half * wchunk], dt_w, tag="w")
                    weng = nc.sync if m % 2 == 0 else nc.scalar
                    weng.dma_start(mw_sb[:], wwT[m])
                for h in range(n_half):
                    if fp8:
                        w_sb, wof = mw_sb, h * wchunk
                    else:
                        w_sb, wof = wpool.tile([128, wchunk], dt, tag="w"), 0
                        weng = (nc.sync if (m * n_half + h) % 2 == 0
                                else nc.scalar)
                        weng.dma_start(
                            w_sb[:], wwT[m][:, h * wchunk:(h + 1) * wchunk])
                    pw_ps = pwp.tile([128, gw], F32, tag="pw")
                    for j in range(KCH):
                        nc.tensor.matmul(
                            pw_ps[:, :],
                            xcol(j, 0, 128),
                            w_sb[:, wof + j * gw:wof + (j + 1) * gw],
                            start=(j == 0), stop=(j == KCH - 1),
                        )
                    if wide:
                        b = gs * C  # 32-row band of this pair
                        nc.vector.tensor_copy(
                            o_sb[h * b:(h + 1) * b, NCLS:],
                            pw_ps[h * b:(h + 1) * b, :])
                    else:
                        for a in range(gs):
                            q = h * gs + a  # slot in m-tile
                            nc.vector.tensor_copy(
                                o_sb[q * C:(q + 1) * C, NCLS:],
                                pw_ps[q * C:(q + 1) * C,
                                      a * CHUNK:(a + 1) * CHUNK])
            else:
                # exact f32: per-slot M=C matmuls col-tiled into one tile
                w_sb = wpool.tile([128, per_mt * KCH * CHUNK], dt, tag="w")
                nc.sync.dma_start(w_sb[:], wwT[m])
                pw_ps = pwp.tile([128, CHUNK], F32, tag="pw")
                for q in range(per_mt):
                    for j in range(KCH):
                        nc.tensor.matmul(
                            pw_ps[q * C:(q + 1) * C, :],
                            xcol(j, q * C, (q + 1) * C),
                            w_sb[:, (q * KCH + j) * CHUNK:
                                 (q * KCH + j + 1) * CHUNK],
                            start=(j == 0), stop=(j == KCH - 1),
                            tile_position=(0, q * C),
                        )
                nc.vector.tensor_copy(o_sb[:, NCLS:], pw_ps[:])

            nc.gpsimd.dma_start(out[m * 128:(m + 1) * 128, :], o_sb[:])

    nc.compile()
    return nc


def _route(cls, mode):
    """Group tokens by class into capacity-padded slots: one slot per class,
    C tokens of capacity.  The (rare) tokens beyond a class's capacity are
    returned as `overflow` and evaluated directly on the host in numpy.

    Returns (C, slots, tok_idx [NCORES, slots*C] int64 token id or -1,
    slot_cls [NCORES, slots] class id per slot, overflow token-id array).
    """
    counts = np.bincount(cls, minlength=NCLS_PAD)
    # coltile (exact f32) needs C to be a multiple of 32 for PSUM col tiling
    cands = ((16, 32, 64, 128) if mode in ("f32r", "bf16", "fp8")
             else (32, 64, 128))
    C = cands[-1]
    for c in cands:
        if int(np.maximum(counts - c, 0).sum()) <= 32:
            C = c
            break

    order = np.argsort(cls, kind="stable")
    starts = np.zeros(NCLS_PAD + 1, np.int64)
    starts[1:] = np.cumsum(counts)

    slots = CPC  # one slot per class owned by the core
    tok_idx = np.full((NCORES, slots * C), -1, np.int64)
    slot_cls = np.full((NCORES, slots), -1, np.int64)
    overflow = []
    for k in range(NCORES):
        for s in range(slots):
            c = k * CPC + s
            lo, cnt = int(starts[c]), int(counts[c])
            n = min(C, cnt)
            slot_cls[k, s] = c
            if n > 0:
                tok_idx[k, s * C:s * C + n] = order[lo:lo + n]
            if cnt > C:
                overflow.append(order[lo + C:lo + cnt])
    overflow = (np.concatenate(overflow) if overflow
                else np.zeros((0,), np.int64))
    return C, slots, tok_idx, slot_cls, overflow


def kernel(x, Wc, bc, Ww, bw, cls_idx, _trace=False, _trace_cores=None,
           _mode=None):
    global LAST_RESULT
    mode = _mode or MODE
    ndt = _NP_DT[mode]
    if ndt is None:
        mode = "f32"
        ndt = np.float32

    x = np.ascontiguousarray(np.asarray(x, np.float32))
    Wc = np.ascontiguousarray(np.asarray(Wc, np.float32))
    bc = np.asarray(bc, np.float32)
    Ww = np.ascontiguousarray(np.asarray(Ww, np.float32))
    bw = np.asarray(bw, np.float32)
    cls = np.asarray(cls_idx).astype(np.int64).ravel()
    N = cls.shape[0]

    C, slots, tok_idx, slot_cls, overflow = _route(cls, mode)
    npad = slots * C
    n_mt = npad // 128
    per_mt = 128 // C
    block = mode in ("f32r", "bf16", "fp8")
    fp8 = mode == "fp8"
    gs = 2 if (block and per_mt >= 2) else 1
    ncls_p = 256 if block else NCLS

    key = (C, slots, mode)
    if key not in _program_cache:
        _program_cache[key] = (_build_program_fp8(C, slots) if fp8
                               else _build_program(C, slots, mode))
    nc = _program_cache[key]

    # wcT [128, KCH*ncls_p]: wcT[p, j*ncls_p+c] = Wc[c, j*128+p]  (replicated)
    Wc_p = Wc if ncls_p == NCLS else np.concatenate(
        [Wc, np.zeros((ncls_p - NCLS, NHID), np.float32)], 0)
    wcT = np.ascontiguousarray(
        Wc_p.reshape(ncls_p, KCH, 128).transpose(2, 1, 0)
            .reshape(128, KCH * ncls_p).astype(ndt))

    Ww_pad = np.zeros((NCLS_PAD, CHUNK, NHID), np.float32)
    Ww_pad[:NCLS] = Ww

    in_maps = []
    for k in range(NCORES):
        # per-slot k-major weights: tmp[s, j, p, w] = Ww[cls_s, w, j*128+p]
        wsel = Ww_pad[np.maximum(slot_cls[k], 0)]
        wsel[slot_cls[k] < 0] = 0.0
        tmp = wsel.reshape(slots, CHUNK, KCH, 128).transpose(0, 2, 3, 1)
        if gs == 2:
            # group = m-tile (per_mt slots); within: pair r, then j, then
            # the two slots' CHUNK columns side by side
            tmp = tmp.reshape(n_mt, per_mt // 2, 2, KCH, 128, CHUNK)
            tmp = tmp.transpose(0, 4, 1, 3, 2, 5)  # [n_mt,128,pair,j,2,CHUNK]
        else:
            tmp = tmp.reshape(n_mt, per_mt, KCH, 128, CHUNK)
            tmp = tmp.transpose(0, 3, 1, 2, 4)     # [n_mt,128,q,j,CHUNK]
        tmp = tmp.reshape(n_mt, 128, per_mt * KCH * CHUNK)
        if fp8:
            wwT = np.ascontiguousarray(
                (tmp * WSCALE).astype(ml_dtypes.float8_e3m4))
        else:
            wwT = np.ascontiguousarray(tmp.astype(ndt))

        ti = tok_idx[k]
        xk = x[np.maximum(ti, 0)]
        xk[ti < 0] = 0.0
        # xT[p, (m*KCH+j)*128 + t] = xk[m*128+t, j*128+p]
        xT = np.ascontiguousarray(
            xk.reshape(n_mt, 128, KCH, 128).transpose(3, 0, 2, 1)
              .reshape(128, n_mt * KCH * 128).astype(ndt))
        in_maps.append({"xT": xT, "wcT": wcT, "wwT": wwT})

    LAST_RESULT = run_bass_kernel_spmd(
        nc, in_maps, list(range(NCORES)), trace=_trace,
        trace_cores=(_trace_cores if _trace else None))

    wide = block and C == 16
    out = np.zeros((N, NCOL), np.float32)
    if wide:
        # row r of a core's output holds its pair's full 2*CHUNK block;
        # slot parity selects which CHUNK half is this row's class
        a_row = (np.arange(npad) // C) % 2
    for k in range(NCORES):
        ok = np.asarray(LAST_RESULT.results[k]["out"]).astype(np.float32)
        if fp8:
            ok[:, NCLS:] *= 1.0 / WSCALE   # undo host prescale of Ww
        if wide:
            words = np.where((a_row == 0)[:, None],
                             ok[:, NCLS:NCLS + CHUNK],
                             ok[:, NCLS + CHUNK:NCLS + 2 * CHUNK])
            ok = np.concatenate([ok[:, :NCLS], words], 1)
        valid = tok_idx[k] >= 0
        out[tok_idx[k][valid]] = ok[valid]

    if overflow.size:
        # rare capacity-overflow tokens: evaluate directly on the host
        xo = x[overflow]                                   # [no, NHID]
        out[overflow, :NCLS] = xo @ Wc.T
        co = cls[overflow]
        out[overflow, NCLS:] = np.einsum(
            "nkh,nh->nk", Ww[co], xo, optimize=True)

    out[:, :NCLS] += bc
    out[:, NCLS:] += bw[cls]
    return out



# revision 17
# speedup vs baseline: 1.2942x; 1.0199x over previous
"""Class-based decoder (MoE-style routing) on 8 trn2 NeuronCores.

Strategy: expert-parallel. Classes are padded 250->256 and split 32 per core.
On the host, tokens are grouped by class into capacity-padded slots (C tokens
per class slot, C in {32,64,128}); class slots that overflow C spill into
extra slots holding a duplicate of the class weights.  Each core receives:
  - xT   [128, n_mt*KCH*128]   its padded tokens, pre-transposed k-major
  - wcT  [128, KCH*NCLS_P]     the (replicated) class-decoder weights, k-major
  - wwT  [n_grp, 128, GRP*KCH*CHUNK]  its word-decoder shard, k-major, grouped
and computes, for every 128-token tile, the class logits (x @ Wc.T) and the
per-class word logits (x_c @ Ww[c].T) as PE matmuls accumulating K=512 over
4 PSUM chunks.  Class slots of a tile are col-tiled into one PSUM tile.
Biases (zero in practice, but handled for correctness) are added on the host
during the final unpermute.

Precision modes:
  f32  : exact fp32 matmuls (2-pass LOW/HIGH on PE; bit-exact, slowest)
  f32r : fp32 data, single-pass PE (TF32-like rounding). Classes are paired
         into N=400 matmuls and NCLS padded to 256 so the free dim is >=256,
         where f32r streams at full rate.
  bf16 : weights/activations cast to bf16 (halves the big W DMA)
  fp8  : word-decoder weights in float8e3 (E3M4), prescaled by 128 so the
         uniform(-0.1,0.1) values land in the normal range; x/Wc stay bf16
         (PE allows mixed non-fp32 operand dtypes) and the output is stored
         bf16.  Halves the dominant W DMA again vs bf16; ~0.9% rel err.
"""

import numpy as np
from contextlib import ExitStack

import concourse.bass as bass
import concourse.bacc as bacc
import concourse.tile as tile
import concourse.mybir as mybir
from concourse.bass_utils import run_bass_kernel_spmd

NHID = 512
NCLS = 250
CHUNK = 200
NCORES = 8
KCH = NHID // 128          # 4 contraction chunks of 128
NCLS_PAD = 256             # classes padded so each core owns an equal shard
CPC = NCLS_PAD // NCORES   # classes per core
NCOL = NCLS + CHUNK        # 450 output columns
F32 = mybir.dt.float32

MODE = "fp8"               # default precision mode; see module docstring
WSCALE = 128.0             # fp8 mode: host prescale of Ww (undone after run)

LAST_RESULT = None         # BassKernelResults of the most recent device run
_program_cache = {}

_MM_DT = {"f32": mybir.dt.float32, "f32r": mybir.dt.float32r,
          "bf16": mybir.dt.bfloat16, "fp8": mybir.dt.bfloat16}
_NP_DT = {"f32": np.float32, "f32r": np.float32, "bf16": None, "fp8": None}
try:
    import ml_dtypes
    _NP_DT["bf16"] = ml_dtypes.bfloat16
    _NP_DT["fp8"] = ml_dtypes.bfloat16   # x/Wc dtype; Ww uses float8_e3m4
except ImportError:
    pass


def _build_program_fp8(C, slots):
    """fp8 SPMD program, restructured for latency:
      - x on sync + wc on scalar first, then all W m-tile chunks (alternating
        queues, fully resident in SBUF: bufs=n_mt) -- the DMA streams never
        stall on compute.
      - a dummy-matmul warmup chain occupies the PE from kernel start so the
        HAM clock gate reaches 8/8 (~3.4us) before the real matmuls arrive.
      - all class matmuls run first (they only need x+wc), then the word
        matmuls chase the W stream m-tile by m-tile.
      - PSUM->SBUF copies are split between DVE and ACT so neither paces the
        word-matmul stream.
    """
    n_mt = (slots * C) // 128
    npad = slots * C
    per_mt = 128 // C
    gs = 2 if per_mt >= 2 else 1
    gw = gs * CHUNK
    n_half = per_mt // gs
    ncls_p = 256
    wide = C == 16
    ocol = NCLS + (gw if wide else CHUNK)
    BF16 = mybir.dt.bfloat16
    FP8 = mybir.dt.float8e3
    wchunk = KCH * gw

    nc = bacc.Bacc("TRN2", target_bir_lowering=False, debug=False,
                   num_devices=NCORES)
    xT = nc.dram_tensor("xT", [128, n_mt * KCH * 128], BF16,
                        kind="ExternalInput")
    wcT = nc.dram_tensor("wcT", [128, KCH * ncls_p], BF16,
                         kind="ExternalInput")
    wwT = nc.dram_tensor("wwT", [n_mt, 128, per_mt * KCH * CHUNK], FP8,
                         kind="ExternalInput")
    out = nc.dram_tensor("out", [npad, ocol], BF16, kind="ExternalOutput")

    hchunk = (n_half // 2) * wchunk  # free-dim elems per half-m-tile W DMA

    with tile.TileContext(nc) as tc, ExitStack() as ctx:
        xpool = ctx.enter_context(tc.tile_pool(name="x", bufs=1))
        wcpool = ctx.enter_context(tc.tile_pool(name="wc", bufs=1))
        wpool = ctx.enter_context(tc.tile_pool(name="w", bufs=2 * n_mt))
        opool = ctx.enter_context(tc.tile_pool(name="o", bufs=n_mt))
        wupool = ctx.enter_context(tc.tile_pool(name="wu", bufs=1))
        pcp = ctx.enter_context(
            tc.tile_pool(name="pc", bufs=2, space=bass.MemorySpace.PSUM))
        pwp = ctx.enter_context(
            tc.tile_pool(name="pw", bufs=5, space=bass.MemorySpace.PSUM))

        # ---- all input DMAs upfront, spread over the two HWDGE queues.
        # W comes as half-m-tile chunks; the first words chunk leads the
        # scalar queue (x leads sync) so words m0 can start ~2us earlier.
        x_sb = xpool.tile([128, n_mt * KCH * 128], BF16)
        wc_sb = wcpool.tile([128, KCH * ncls_p], BF16)
        w_sbs = [[None, None] for _ in range(n_mt)]

        def wdma(eng, m, half):
            w_sb = wpool.tile([128, hchunk], FP8, tag="w")
            eng.dma_start(w_sb[:],
                          wwT[m][:, half * hchunk:(half + 1) * hchunk])
            w_sbs[m][half] = w_sb

        nc.sync.dma_start(wc_sb[:], wcT[:])
        nc.sync.dma_start(x_sb[:], xT[:])
        for m in range(n_mt):
            wdma(nc.scalar, m, 0)   # first halves: words h0/h1 of each m-tile
        for m in range(n_mt):
            wdma(nc.sync, m, 1)     # second halves trail on the sync queue

        # ---- PE warmup: dummy matmuls so HAM hits 8/8 before real work ----
        wu_sb = wupool.tile([128, 384], BF16)
        nc.vector.memset(wu_sb[:], 0)
        wu_ps = pwp.tile([128, gw], F32, tag="pw")
        for i in range(12):
            nc.tensor.matmul(wu_ps[:, :384], wu_sb[:, :128], wu_sb[:, :384],
                             start=True, stop=True)

        def xcol(m, j):
            base = (m * KCH + j) * 128
            return x_sb[:, base:base + 128]

        # ---- per m-tile: class logits (x+wc only), then words as W lands --
        for m in range(n_mt):
            pc_ps = pcp.tile([128, ncls_p], F32)
            for j in range(KCH):
                nc.tensor.matmul(
                    pc_ps[:, :], xcol(m, j),
                    wc_sb[:, j * ncls_p:(j + 1) * ncls_p],
                    start=(j == 0), stop=(j == KCH - 1))
            o_sb = opool.tile([128, ocol], BF16)
            nc.scalar.copy(o_sb[:, :NCLS], pc_ps[:, :NCLS])

            for h in range(n_half):
                w_sb = w_sbs[m][h // (n_half // 2)]
                wof = (h % (n_half // 2)) * wchunk
                pw_ps = pwp.tile([128, gw], F32, tag="pw")
                for j in range(KCH):
                    nc.tensor.matmul(
                        pw_ps[:, :], xcol(m, j),
                        w_sb[:, wof + j * gw:wof + (j + 1) * gw],
                        start=(j == 0), stop=(j == KCH - 1))
                ceng = nc.vector.tensor_copy if h % 2 == 0 else nc.scalar.copy
                if wide:
                    b = gs * C
                    ceng(o_sb[h * b:(h + 1) * b, NCLS:],
                         pw_ps[h * b:(h + 1) * b, :])
                else:
                    for a in range(gs):
                        q = h * gs + a
                        ceng(o_sb[q * C:(q + 1) * C, NCLS:],
                             pw_ps[q * C:(q + 1) * C,
                                   a * CHUNK:(a + 1) * CHUNK])
            # last store on the (by now idle) sync HWDGE ring: cheaper
            # completion than SWDGE, so the teardown drain starts earlier
            seng = nc.sync if m == n_mt - 1 else nc.gpsimd
            seng.dma_start(out[m * 128:(m + 1) * 128, :], o_sb[:])

    nc.compile()
    return nc


def _build_program(C, slots, mode):
    """One SPMD program: slots class-slots of C tokens each, per core.

    f32 uses the "coltile" scheme: per class slot, an M=C matmul col-tiled
    into a shared PSUM tile (exact 2-pass fp32).
    f32r/bf16 use the "block" scheme: every matmul is M=128 (all slots of an
    m-tile), and the word logits come as per_mt//2 halves of N=2*CHUNK whose
    off-diagonal class blocks are discarded by the PSUM->SBUF copies.  This
    keeps N>=256 (full-rate f32r) and NumWeights=128 (FWL weight loads).
    """
    n_mt = (slots * C) // 128  # 128-token m-tiles
    npad = slots * C
    per_mt = 128 // C          # class slots per m-tile
    block = mode in ("f32r", "bf16", "fp8")
    fp8 = mode == "fp8"
    # class slots per pw matmul and word-columns per pw matmul
    gs = 2 if (block and per_mt >= 2) else 1
    gw = gs * CHUNK            # 400 paired / 200 single
    n_half = per_mt // gs      # pw matmul groups per m-tile
    ncls_p = 256 if block else NCLS  # N>=256 keeps f32r at full rate
    # C=16 diag copies would need 16-partition bases (illegal); store each
    # 32-row band's full pair block instead and let the host pick the diagonal
    wide = block and C == 16
    ocol = NCLS + (gw if wide else CHUNK)
    dt = _MM_DT[mode]
    dt_w = mybir.dt.float8e3 if fp8 else dt
    dt_o = mybir.dt.bfloat16 if fp8 else F32

    nc = bacc.Bacc("TRN2", target_bir_lowering=False, debug=False,
                   num_devices=NCORES)
    xT = nc.dram_tensor("xT", [128, n_mt * KCH * 128], dt, kind="ExternalInput")
    wcT = nc.dram_tensor("wcT", [128, KCH * ncls_p], dt, kind="ExternalInput")
    # W groups: one DMA per m-tile worth of class slots
    wwT = nc.dram_tensor("wwT", [n_mt, 128, per_mt * KCH * CHUNK], dt_w,
                         kind="ExternalInput")
    out = nc.dram_tensor("out", [npad, ocol], dt_o, kind="ExternalOutput")

    with tile.TileContext(nc) as tc, ExitStack() as ctx:
        xpool = ctx.enter_context(tc.tile_pool(name="x", bufs=1))
        wcpool = ctx.enter_context(tc.tile_pool(name="wc", bufs=1))
        wpool = ctx.enter_context(tc.tile_pool(name="w", bufs=(3 if fp8 else 12)))
        opool = ctx.enter_context(tc.tile_pool(name="o", bufs=8))
        pcp = ctx.enter_context(
            tc.tile_pool(name="pc", bufs=2, space=bass.MemorySpace.PSUM))
        pwp = ctx.enter_context(
            tc.tile_pool(name="pw", bufs=6, space=bass.MemorySpace.PSUM))

        # three independent DMA streams so nothing blocks the W firehose:
        #   sync (SP HWDGE): only the big W chunks, back to back
        #   scalar (ACT HWDGE): wc + per-m-tile x loads
        #   gpsimd (SWDGE): output stores
        wc_sb = wcpool.tile([128, KCH * ncls_p], dt)
        x_sb = xpool.tile([128, n_mt * KCH * 128], dt)
        if fp8:
            # one big x DMA upfront (x is small; W chunks then own the queues)
            nc.scalar.dma_start(x_sb[:], xT[:])
        nc.scalar.dma_start(wc_sb[:], wcT[:])

        wchunk = KCH * gw  # free-dim elems per W DMA (one pw matmul group)
        for m in range(n_mt):
            if not fp8:
                # x columns for this m-tile: [(m*KCH+j)*128 + t]
                nc.scalar.dma_start(x_sb[:, m * KCH * 128:(m + 1) * KCH * 128],
                                    xT[:, m * KCH * 128:(m + 1) * KCH * 128])

            def xcol(j, lo, hi):
                base = (m * KCH + j) * 128
                return x_sb[:, base + lo:base + hi]

            # class logits for these 128 tokens
            pc_ps = pcp.tile([128, ncls_p], F32)
            for j in range(KCH):
                nc.tensor.matmul(
                    pc_ps[:, :],
                    xcol(j, 0, 128),
                    wc_sb[:, j * ncls_p:(j + 1) * ncls_p],
                    start=(j == 0), stop=(j == KCH - 1),
                )

            o_sb = opool.tile([128, ocol], F32)
            nc.vector.tensor_copy(o_sb[:, :NCLS], pc_ps[:, :NCLS])

            if block:
                # word logits: per half, one M=128 matmul of N=gw covering
                # gs classes; only each slot's own class block is kept
                mw_sb = None
                if fp8:
                    # one big W DMA per m-tile (4x fewer, larger transfers)
                    mw_sb = wpool.tile([128, n_half * wchunk], dt_w, tag="w")
                    weng = nc.sync if m % 2 == 0 else nc.scalar
                    weng.dma_start(mw_sb[:], wwT[m])
                for h in range(n_half):
                    if fp8:
                        w_sb, wof = mw_sb, h * wchunk
                    else:
                        w_sb, wof = wpool.tile([128, wchunk], dt, tag="w"), 0
                        weng = (nc.sync if (m * n_half + h) % 2 == 0
                                else nc.scalar)
                        weng.dma_start(
                            w_sb[:], wwT[m][:, h * wchunk:(h + 1) * wchunk])
                    pw_ps = pwp.tile([128, gw], F32, tag="pw")
                    for j in range(KCH):
                        nc.tensor.matmul(
                            pw_ps[:, :],
                            xcol(j, 0, 128),
                            w_sb[:, wof + j * gw:wof + (j + 1) * gw],
                            start=(j == 0), stop=(j == KCH - 1),
                        )
                    if wide:
                        b = gs * C  # 32-row band of this pair
                        nc.vector.tensor_copy(
                            o_sb[h * b:(h + 1) * b, NCLS:],
                            pw_ps[h * b:(h + 1) * b, :])
                    else:
                        for a in range(gs):
                            q = h * gs + a  # slot in m-tile
                            nc.vector.tensor_copy(
                                o_sb[q * C:(q + 1) * C, NCLS:],
                                pw_ps[q * C:(q + 1) * C,
                                      a * CHUNK:(a + 1) * CHUNK])
            else:
                # exact f32: per-slot M=C matmuls col-tiled into one tile
                w_sb = wpool.tile([128, per_mt * KCH * CHUNK], dt, tag="w")
                nc.sync.dma_start(w_sb[:], wwT[m])
                pw_ps = pwp.tile([128, CHUNK], F32, tag="pw")
                for q in range(per_mt):
                    for j in range(KCH):
                        nc.tensor.matmul(
                            pw_ps[q * C:(q + 1) * C, :],
                            xcol(j, q * C, (q + 1) * C),
                            w_sb[:, (q * KCH + j) * CHUNK:
                                 (q * KCH + j + 1) * CHUNK],
                            start=(j == 0), stop=(j == KCH - 1),
                            tile_position=(0, q * C),
                        )
                nc.vector.tensor_copy(o_sb[:, NCLS:], pw_ps[:])

            nc.gpsimd.dma_start(out[m * 128:(m + 1) * 128, :], o_sb[:])

    nc.compile()
    return nc


def _route(cls, mode):
    """Group tokens by class into capacity-padded slots: one slot per class,
    C tokens of capacity.  The (rare) tokens beyond a class's capacity are
    returned as `overflow` and evaluated directly on the host in numpy.

    Returns (C, slots, tok_idx [NCORES, slots*C] int64 token id or -1,
    slot_cls [NCORES, slots] class id per slot, overflow token-id array).
    """
    counts = np.bincount(cls, minlength=NCLS_PAD)
    # coltile (exact f32) needs C to be a multiple of 32 for PSUM col tiling
    cands = ((16, 32, 64, 128) if mode in ("f32r", "bf16", "fp8")
             else (32, 64, 128))
    C = cands[-1]
    for c in cands:
        if int(np.maximum(counts - c, 0).sum()) <= 32:
            C = c
            break

    order = np.argsort(cls, kind="stable")
    starts = np.zeros(NCLS_PAD + 1, np.int64)
    starts[1:] = np.cumsum(counts)

    slots = CPC  # one slot per class owned by the core
    tok_idx = np.full((NCORES, slots * C), -1, np.int64)
    slot_cls = np.full((NCORES, slots), -1, np.int64)
    overflow = []
    for k in range(NCORES):
        for s in range(slots):
            c = k * CPC + s
            lo, cnt = int(starts[c]), int(counts[c])
            n = min(C, cnt)
            slot_cls[k, s] = c
            if n > 0:
                tok_idx[k, s * C:s * C + n] = order[lo:lo + n]
            if cnt > C:
                overflow.append(order[lo + C:lo + cnt])
    overflow = (np.concatenate(overflow) if overflow
                else np.zeros((0,), np.int64))
    return C, slots, tok_idx, slot_cls, overflow


def kernel(x, Wc, bc, Ww, bw, cls_idx, _trace=False, _trace_cores=None,
           _mode=None):
    global LAST_RESULT
    mode = _mode or MODE
    ndt = _NP_DT[mode]
    if ndt is None:
        mode = "f32"
        ndt = np.float32

    x = np.ascontiguousarray(np.asarray(x, np.float32))
    Wc = np.ascontiguousarray(np.asarray(Wc, np.float32))
    bc = np.asarray(bc, np.float32)
    Ww = np.ascontiguousarray(np.asarray(Ww, np.float32))
    bw = np.asarray(bw, np.float32)
    cls = np.asarray(cls_idx).astype(np.int64).ravel()
    N = cls.shape[0]

    C, slots, tok_idx, slot_cls, overflow = _route(cls, mode)
    npad = slots * C
    n_mt = npad // 128
    per_mt = 128 // C
    block = mode in ("f32r", "bf16", "fp8")
    fp8 = mode == "fp8"
    gs = 2 if (block and per_mt >= 2) else 1
    ncls_p = 256 if block else NCLS

    key = (C, slots, mode)
    if key not in _program_cache:
        _program_cache[key] = (_build_program_fp8(C, slots) if fp8
                               else _build_program(C, slots, mode))
    nc = _program_cache[key]

    # wcT [128, KCH*ncls_p]: wcT[p, j*ncls_p+c] = Wc[c, j*128+p]  (replicated)
    Wc_p = Wc if ncls_p == NCLS else np.concatenate(
        [Wc, np.zeros((ncls_p - NCLS, NHID), np.float32)], 0)
    wcT = np.ascontiguousarray(
        Wc_p.reshape(ncls_p, KCH, 128).transpose(2, 1, 0)
            .reshape(128, KCH * ncls_p).astype(ndt))

    Ww_pad = np.zeros((NCLS_PAD, CHUNK, NHID), np.float32)
    Ww_pad[:NCLS] = Ww

    in_maps = []
    for k in range(NCORES):
        # per-slot k-major weights: tmp[s, j, p, w] = Ww[cls_s, w, j*128+p]
        wsel = Ww_pad[np.maximum(slot_cls[k], 0)]
        wsel[slot_cls[k] < 0] = 0.0
        tmp = wsel.reshape(slots, CHUNK, KCH, 128).transpose(0, 2, 3, 1)
        if gs == 2:
            # group = m-tile (per_mt slots); within: pair r, then j, then
            # the two slots' CHUNK columns side by side
            tmp = tmp.reshape(n_mt, per_mt // 2, 2, KCH, 128, CHUNK)
            tmp = tmp.transpose(0, 4, 1, 3, 2, 5)  # [n_mt,128,pair,j,2,CHUNK]
        else:
            tmp = tmp.reshape(n_mt, per_mt, KCH, 128, CHUNK)
            tmp = tmp.transpose(0, 3, 1, 2, 4)     # [n_mt,128,q,j,CHUNK]
        tmp = tmp.reshape(n_mt, 128, per_mt * KCH * CHUNK)
        if fp8:
            wwT = np.ascontiguousarray(
                (tmp * WSCALE).astype(ml_dtypes.float8_e3m4))
        else:
            wwT = np.ascontiguousarray(tmp.astype(ndt))

        ti = tok_idx[k]
        xk = x[np.maximum(ti, 0)]
        xk[ti < 0] = 0.0
        # xT[p, (m*KCH+j)*128 + t] = xk[m*128+t, j*128+p]
        xT = np.ascontiguousarray(
            xk.reshape(n_mt, 128, KCH, 128).transpose(3, 0, 2, 1)
              .reshape(128, n_mt * KCH * 128).astype(ndt))
        in_maps.append({"xT": xT, "wcT": wcT, "wwT": wwT})

    LAST_RESULT = run_bass_kernel_spmd(
        nc, in_maps, list(range(NCORES)), trace=_trace,
        trace_cores=(_trace_cores if _trace else None))

    wide = block and C == 16
    out = np.zeros((N, NCOL), np.float32)
    if wide:
        # row r of a core's output holds its pair's full 2*CHUNK block;
        # slot parity selects which CHUNK half is this row's class
        a_row = (np.arange(npad) // C) % 2
    for k in range(NCORES):
        ok = np.asarray(LAST_RESULT.results[k]["out"]).astype(np.float32)
        if fp8:
            ok[:, NCLS:] *= 1.0 / WSCALE   # undo host prescale of Ww
        if wide:
            words = np.where((a_row == 0)[:, None],
                             ok[:, NCLS:NCLS + CHUNK],
                             ok[:, NCLS + CHUNK:NCLS + 2 * CHUNK])
            ok = np.concatenate([ok[:, :NCLS], words], 1)
        valid = tok_idx[k] >= 0
        out[tok_idx[k][valid]] = ok[valid]

    if overflow.size:
        # rare capacity-overflow tokens: evaluate directly on the host
        xo = x[overflow]                                   # [no, NHID]
        out[overflow, :NCLS] = xo @ Wc.T
        co = cls[overflow]
        out[overflow, NCLS:] = np.einsum(
            "nkh,nh->nk", Ww[co], xo, optimize=True)

    out[:, :NCLS] += bc
    out[:, NCLS:] += bw[cls]
    return out



# revision 18
# speedup vs baseline: 1.2995x; 1.0041x over previous
"""Class-based decoder (MoE-style routing) on 8 trn2 NeuronCores.

Strategy: expert-parallel. Classes are padded 250->256 and split 32 per core.
On the host, tokens are grouped by class into capacity-padded slots (C tokens
per class slot, C in {32,64,128}); class slots that overflow C spill into
extra slots holding a duplicate of the class weights.  Each core receives:
  - xT   [128, n_mt*KCH*128]   its padded tokens, pre-transposed k-major
  - wcT  [128, KCH*NCLS_P]     the (replicated) class-decoder weights, k-major
  - wwT  [n_grp, 128, GRP*KCH*CHUNK]  its word-decoder shard, k-major, grouped
and computes, for every 128-token tile, the class logits (x @ Wc.T) and the
per-class word logits (x_c @ Ww[c].T) as PE matmuls accumulating K=512 over
4 PSUM chunks.  Class slots of a tile are col-tiled into one PSUM tile.
Biases (zero in practice, but handled for correctness) are added on the host
during the final unpermute.

Precision modes:
  f32  : exact fp32 matmuls (2-pass LOW/HIGH on PE; bit-exact, slowest)
  f32r : fp32 data, single-pass PE (TF32-like rounding). Classes are paired
         into N=400 matmuls and NCLS padded to 256 so the free dim is >=256,
         where f32r streams at full rate.
  bf16 : weights/activations cast to bf16 (halves the big W DMA)
  fp8  : word-decoder weights in float8e3 (E3M4), prescaled by 128 so the
         uniform(-0.1,0.1) values land in the normal range; x/Wc stay bf16
         (PE allows mixed non-fp32 operand dtypes) and the output is stored
         bf16.  Halves the dominant W DMA again vs bf16; ~0.9% rel err.
"""

import numpy as np
from contextlib import ExitStack

import concourse.bass as bass
import concourse.bacc as bacc
import concourse.tile as tile
import concourse.mybir as mybir
from concourse.bass_utils import run_bass_kernel_spmd

NHID = 512
NCLS = 250
CHUNK = 200
NCORES = 8
KCH = NHID // 128          # 4 contraction chunks of 128
NCLS_PAD = 256             # classes padded so each core owns an equal shard
CPC = NCLS_PAD // NCORES   # classes per core
NCOL = NCLS + CHUNK        # 450 output columns
F32 = mybir.dt.float32

MODE = "fp8"               # default precision mode; see module docstring
WSCALE = 128.0             # fp8 mode: host prescale of Ww (undone after run)

LAST_RESULT = None         # BassKernelResults of the most recent device run
_program_cache = {}

_MM_DT = {"f32": mybir.dt.float32, "f32r": mybir.dt.float32r,
          "bf16": mybir.dt.bfloat16, "fp8": mybir.dt.bfloat16}
_NP_DT = {"f32": np.float32, "f32r": np.float32, "bf16": None, "fp8": None}
try:
    import ml_dtypes
    _NP_DT["bf16"] = ml_dtypes.bfloat16
    _NP_DT["fp8"] = ml_dtypes.bfloat16   # x/Wc dtype; Ww uses float8_e3m4
except ImportError:
    pass


def _build_program_fp8(C, slots):
    """fp8 SPMD program, restructured for latency:
      - x on sync + wc on scalar first, then all W m-tile chunks (alternating
        queues, fully resident in SBUF: bufs=n_mt) -- the DMA streams never
        stall on compute.
      - a dummy-matmul warmup chain occupies the PE from kernel start so the
        HAM clock gate reaches 8/8 (~3.4us) before the real matmuls arrive.
      - all class matmuls run first (they only need x+wc), then the word
        matmuls chase the W stream m-tile by m-tile.
      - PSUM->SBUF copies are split between DVE and ACT so neither paces the
        word-matmul stream.
    """
    n_mt = (slots * C) // 128
    npad = slots * C
    per_mt = 128 // C
    gs = 2 if per_mt >= 2 else 1
    gw = gs * CHUNK
    n_half = per_mt // gs
    ncls_p = 256
    wide = C == 16
    ocol = NCLS + (gw if wide else CHUNK)
    BF16 = mybir.dt.bfloat16
    FP8 = mybir.dt.float8e3
    wchunk = KCH * gw

    nc = bacc.Bacc("TRN2", target_bir_lowering=False, debug=False,
                   num_devices=NCORES)
    xT = nc.dram_tensor("xT", [128, n_mt * KCH * 128], BF16,
                        kind="ExternalInput")
    wcT = nc.dram_tensor("wcT", [128, KCH * ncls_p], BF16,
                         kind="ExternalInput")
    wwT = nc.dram_tensor("wwT", [n_mt, 128, per_mt * KCH * CHUNK], FP8,
                         kind="ExternalInput")
    out = nc.dram_tensor("out", [npad, ocol], BF16, kind="ExternalOutput")

    hchunk = (n_half // 2) * wchunk  # free-dim elems per half-m-tile W DMA

    with tile.TileContext(nc) as tc, ExitStack() as ctx:
        xpool = ctx.enter_context(tc.tile_pool(name="x", bufs=1))
        wcpool = ctx.enter_context(tc.tile_pool(name="wc", bufs=1))
        wpool = ctx.enter_context(tc.tile_pool(name="w", bufs=2 * n_mt))
        opool = ctx.enter_context(tc.tile_pool(name="o", bufs=n_mt))
        wupool = ctx.enter_context(tc.tile_pool(name="wu", bufs=1))
        pcp = ctx.enter_context(
            tc.tile_pool(name="pc", bufs=2, space=bass.MemorySpace.PSUM))
        pwp = ctx.enter_context(
            tc.tile_pool(name="pw", bufs=5, space=bass.MemorySpace.PSUM))

        # ---- all input DMAs upfront, spread over the two HWDGE queues.
        # W comes as half-m-tile chunks; the first words chunk leads the
        # scalar queue (x leads sync) so words m0 can start ~2us earlier.
        x_sb = xpool.tile([128, n_mt * KCH * 128], BF16)
        wc_sb = wcpool.tile([128, KCH * ncls_p], BF16)
        w_sbs = [[None, None] for _ in range(n_mt)]

        def wdma(eng, m, half):
            w_sb = wpool.tile([128, hchunk], FP8, tag="w")
            eng.dma_start(w_sb[:],
                          wwT[m][:, half * hchunk:(half + 1) * hchunk])
            w_sbs[m][half] = w_sb

        # small upfront loads: wc + per-m-tile x chunks.  Small DMAs clear
        # their ~1.5-2us completion receipt much earlier than one big load,
        # which is what gates the first real matmul.
        nc.sync.dma_start(wc_sb[:], wcT[:])
        for m in range(n_mt):
            nc.sync.dma_start(x_sb[:, m * KCH * 128:(m + 1) * KCH * 128],
                              xT[:, m * KCH * 128:(m + 1) * KCH * 128])
        for m in range(n_mt):
            wdma(nc.scalar, m, 0)   # first halves: words h0/h1 of each m-tile
        for m in range(n_mt):
            wdma(nc.sync, m, 1)     # second halves trail on the sync queue

        # ---- PE warmup: dummy matmuls so HAM hits 8/8 before real work.
        # Must hand off to the real matmuls with NO gap: a PE-idle gap
        # resets the HAM activity window and costs ~3.4us of cold clocks.
        wu_sb = wupool.tile([128, 384], BF16)
        nc.vector.memset(wu_sb[:], 0)
        wu_ps = pwp.tile([128, gw], F32, tag="pw")
        for i in range(11):
            nc.tensor.matmul(wu_ps[:, :384], wu_sb[:, :128], wu_sb[:, :384],
                             start=True, stop=True)

        def xcol(m, j):
            base = (m * KCH + j) * 128
            return x_sb[:, base:base + 128]

        # ---- all class logits first (need only x+wc, cover the W wait) ----
        o_sbs = []
        for m in range(n_mt):
            pc_ps = pcp.tile([128, ncls_p], F32)
            for j in range(KCH):
                nc.tensor.matmul(
                    pc_ps[:, :], xcol(m, j),
                    wc_sb[:, j * ncls_p:(j + 1) * ncls_p],
                    start=(j == 0), stop=(j == KCH - 1))
            o_sb = opool.tile([128, ocol], BF16)
            nc.scalar.copy(o_sb[:, :NCLS], pc_ps[:, :NCLS])
            o_sbs.append(o_sb)

        # ---- words per m-tile; each 128/n_half-row band is stored as soon
        # as its copy lands, so the final store is a small late transfer ----
        brows = 128 // n_half
        for m in range(n_mt):
            o_sb = o_sbs[m]
            for h in range(n_half):
                w_sb = w_sbs[m][h // (n_half // 2)]
                wof = (h % (n_half // 2)) * wchunk
                pw_ps = pwp.tile([128, gw], F32, tag="pw")
                for j in range(KCH):
                    nc.tensor.matmul(
                        pw_ps[:, :], xcol(m, j),
                        w_sb[:, wof + j * gw:wof + (j + 1) * gw],
                        start=(j == 0), stop=(j == KCH - 1))
                ceng = nc.vector.tensor_copy if h % 2 == 0 else nc.scalar.copy
                if wide:
                    b = gs * C
                    ceng(o_sb[h * b:(h + 1) * b, NCLS:],
                         pw_ps[h * b:(h + 1) * b, :])
                else:
                    for a in range(gs):
                        q = h * gs + a
                        ceng(o_sb[q * C:(q + 1) * C, NCLS:],
                             pw_ps[q * C:(q + 1) * C,
                                   a * CHUNK:(a + 1) * CHUNK])
                last = m == n_mt - 1 and h == n_half - 1
                seng = nc.sync if last else nc.gpsimd
                r0 = m * 128 + h * brows
                seng.dma_start(out[r0:r0 + brows, :],
                               o_sb[h * brows:(h + 1) * brows, :])

    nc.compile()
    return nc


def _build_program(C, slots, mode):
    """One SPMD program: slots class-slots of C tokens each, per core.

    f32 uses the "coltile" scheme: per class slot, an M=C matmul col-tiled
    into a shared PSUM tile (exact 2-pass fp32).
    f32r/bf16 use the "block" scheme: every matmul is M=128 (all slots of an
    m-tile), and the word logits come as per_mt//2 halves of N=2*CHUNK whose
    off-diagonal class blocks are discarded by the PSUM->SBUF copies.  This
    keeps N>=256 (full-rate f32r) and NumWeights=128 (FWL weight loads).
    """
    n_mt = (slots * C) // 128  # 128-token m-tiles
    npad = slots * C
    per_mt = 128 // C          # class slots per m-tile
    block = mode in ("f32r", "bf16", "fp8")
    fp8 = mode == "fp8"
    # class slots per pw matmul and word-columns per pw matmul
    gs = 2 if (block and per_mt >= 2) else 1
    gw = gs * CHUNK            # 400 paired / 200 single
    n_half = per_mt // gs      # pw matmul groups per m-tile
    ncls_p = 256 if block else NCLS  # N>=256 keeps f32r at full rate
    # C=16 diag copies would need 16-partition bases (illegal); store each
    # 32-row band's full pair block instead and let the host pick the diagonal
    wide = block and C == 16
    ocol = NCLS + (gw if wide else CHUNK)
    dt = _MM_DT[mode]
    dt_w = mybir.dt.float8e3 if fp8 else dt
    dt_o = mybir.dt.bfloat16 if fp8 else F32

    nc = bacc.Bacc("TRN2", target_bir_lowering=False, debug=False,
                   num_devices=NCORES)
    xT = nc.dram_tensor("xT", [128, n_mt * KCH * 128], dt, kind="ExternalInput")
    wcT = nc.dram_tensor("wcT", [128, KCH * ncls_p], dt, kind="ExternalInput")
    # W groups: one DMA per m-tile worth of class slots
    wwT = nc.dram_tensor("wwT", [n_mt, 128, per_mt * KCH * CHUNK], dt_w,
                         kind="ExternalInput")
    out = nc.dram_tensor("out", [npad, ocol], dt_o, kind="ExternalOutput")

    with tile.TileContext(nc) as tc, ExitStack() as ctx:
        xpool = ctx.enter_context(tc.tile_pool(name="x", bufs=1))
        wcpool = ctx.enter_context(tc.tile_pool(name="wc", bufs=1))
        wpool = ctx.enter_context(tc.tile_pool(name="w", bufs=(3 if fp8 else 12)))
        opool = ctx.enter_context(tc.tile_pool(name="o", bufs=8))
        pcp = ctx.enter_context(
            tc.tile_pool(name="pc", bufs=2, space=bass.MemorySpace.PSUM))
        pwp = ctx.enter_context(
            tc.tile_pool(name="pw", bufs=6, space=bass.MemorySpace.PSUM))

        # three independent DMA streams so nothing blocks the W firehose:
        #   sync (SP HWDGE): only the big W chunks, back to back
        #   scalar (ACT HWDGE): wc + per-m-tile x loads
        #   gpsimd (SWDGE): output stores
        wc_sb = wcpool.tile([128, KCH * ncls_p], dt)
        x_sb = xpool.tile([128, n_mt * KCH * 128], dt)
        if fp8:
            # one big x DMA upfront (x is small; W chunks then own the queues)
            nc.scalar.dma_start(x_sb[:], xT[:])
        nc.scalar.dma_start(wc_sb[:], wcT[:])

        wchunk = KCH * gw  # free-dim elems per W DMA (one pw matmul group)
        for m in range(n_mt):
            if not fp8:
                # x columns for this m-tile: [(m*KCH+j)*128 + t]
                nc.scalar.dma_start(x_sb[:, m * KCH * 128:(m + 1) * KCH * 128],
                                    xT[:, m * KCH * 128:(m + 1) * KCH * 128])

            def xcol(j, lo, hi):
                base = (m * KCH + j) * 128
                return x_sb[:, base + lo:base + hi]

            # class logits for these 128 tokens
            pc_ps = pcp.tile([128, ncls_p], F32)
            for j in range(KCH):
                nc.tensor.matmul(
                    pc_ps[:, :],
                    xcol(j, 0, 128),
                    wc_sb[:, j * ncls_p:(j + 1) * ncls_p],
                    start=(j == 0), stop=(j == KCH - 1),
                )

            o_sb = opool.tile([128, ocol], F32)
            nc.vector.tensor_copy(o_sb[:, :NCLS], pc_ps[:, :NCLS])

            if block:
                # word logits: per half, one M=128 matmul of N=gw covering
                # gs classes; only each slot's own class block is kept
                mw_sb = None
                if fp8:
                    # one big W DMA per m-tile (4x fewer, larger transfers)
                    mw_sb = wpool.tile([128, n_half * wchunk], dt_w, tag="w")
                    weng = nc.sync if m % 2 == 0 else nc.scalar
                    weng.dma_start(mw_sb[:], wwT[m])
                for h in range(n_half):
                    if fp8:
                        w_sb, wof = mw_sb, h * wchunk
                    else:
                        w_sb, wof = wpool.tile([128, wchunk], dt, tag="w"), 0
                        weng = (nc.sync if (m * n_half + h) % 2 == 0
                                else nc.scalar)
                        weng.dma_start(
                            w_sb[:], wwT[m][:, h * wchunk:(h + 1) * wchunk])
                    pw_ps = pwp.tile([128, gw], F32, tag="pw")
                    for j in range(KCH):
                        nc.tensor.matmul(
                            pw_ps[:, :],
                            xcol(j, 0, 128),
                            w_sb[:, wof + j * gw:wof + (j + 1) * gw],
                            start=(j == 0), stop=(j == KCH - 1),
                        )
                    if wide:
                        b = gs * C  # 32-row band of this pair
                        nc.vector.tensor_copy(
                            o_sb[h * b:(h + 1) * b, NCLS:],
                            pw_ps[h * b:(h + 1) * b, :])
                    else:
                        for a in range(gs):
                            q = h * gs + a  # slot in m-tile
                            nc.vector.tensor_copy(
                                o_sb[q * C:(q + 1) * C, NCLS:],
                                pw_ps[q * C:(q + 1) * C,
                                      a * CHUNK:(a + 1) * CHUNK])
            else:
                # exact f32: per-slot M=C matmuls col-tiled into one tile
                w_sb = wpool.tile([128, per_mt * KCH * CHUNK], dt, tag="w")
                nc.sync.dma_start(w_sb[:], wwT[m])
                pw_ps = pwp.tile([128, CHUNK], F32, tag="pw")
                for q in range(per_mt):
                    for j in range(KCH):
                        nc.tensor.matmul(
                            pw_ps[q * C:(q + 1) * C, :],
                            xcol(j, q * C, (q + 1) * C),
                            w_sb[:, (q * KCH + j) * CHUNK:
                                 (q * KCH + j + 1) * CHUNK],
                            start=(j == 0), stop=(j == KCH - 1),
                            tile_position=(0, q * C),
                        )
                nc.vector.tensor_copy(o_sb[:, NCLS:], pw_ps[:])

            nc.gpsimd.dma_start(out[m * 128:(m + 1) * 128, :], o_sb[:])

    nc.compile()
    return nc


def _route(cls, mode):
    """Group tokens by class into capacity-padded slots: one slot per class,
    C tokens of capacity.  The (rare) tokens beyond a class's capacity are
    returned as `overflow` and evaluated directly on the host in numpy.

    Returns (C, slots, tok_idx [NCORES, slots*C] int64 token id or -1,
    slot_cls [NCORES, slots] class id per slot, overflow token-id array).
    """
    counts = np.bincount(cls, minlength=NCLS_PAD)
    # coltile (exact f32) needs C to be a multiple of 32 for PSUM col tiling
    cands = ((16, 32, 64, 128) if mode in ("f32r", "bf16", "fp8")
             else (32, 64, 128))
    C = cands[-1]
    for c in cands:
        if int(np.maximum(counts - c, 0).sum()) <= 32:
            C = c
            break

    order = np.argsort(cls, kind="stable")
    starts = np.zeros(NCLS_PAD + 1, np.int64)
    starts[1:] = np.cumsum(counts)

    slots = CPC  # one slot per class owned by the core
    tok_idx = np.full((NCORES, slots * C), -1, np.int64)
    slot_cls = np.full((NCORES, slots), -1, np.int64)
    overflow = []
    for k in range(NCORES):
        for s in range(slots):
            c = k * CPC + s
            lo, cnt = int(starts[c]), int(counts[c])
            n = min(C, cnt)
            slot_cls[k, s] = c
            if n > 0:
                tok_idx[k, s * C:s * C + n] = order[lo:lo + n]
            if cnt > C:
                overflow.append(order[lo + C:lo + cnt])
    overflow = (np.concatenate(overflow) if overflow
                else np.zeros((0,), np.int64))
    return C, slots, tok_idx, slot_cls, overflow


def kernel(x, Wc, bc, Ww, bw, cls_idx, _trace=False, _trace_cores=None,
           _mode=None):
    global LAST_RESULT
    mode = _mode or MODE
    ndt = _NP_DT[mode]
    if ndt is None:
        mode = "f32"
        ndt = np.float32

    x = np.ascontiguousarray(np.asarray(x, np.float32))
    Wc = np.ascontiguousarray(np.asarray(Wc, np.float32))
    bc = np.asarray(bc, np.float32)
    Ww = np.ascontiguousarray(np.asarray(Ww, np.float32))
    bw = np.asarray(bw, np.float32)
    cls = np.asarray(cls_idx).astype(np.int64).ravel()
    N = cls.shape[0]

    C, slots, tok_idx, slot_cls, overflow = _route(cls, mode)
    npad = slots * C
    n_mt = npad // 128
    per_mt = 128 // C
    block = mode in ("f32r", "bf16", "fp8")
    fp8 = mode == "fp8"
    gs = 2 if (block and per_mt >= 2) else 1
    ncls_p = 256 if block else NCLS

    key = (C, slots, mode)
    if key not in _program_cache:
        _program_cache[key] = (_build_program_fp8(C, slots) if fp8
                               else _build_program(C, slots, mode))
    nc = _program_cache[key]

    # wcT [128, KCH*ncls_p]: wcT[p, j*ncls_p+c] = Wc[c, j*128+p]  (replicated)
    Wc_p = Wc if ncls_p == NCLS else np.concatenate(
        [Wc, np.zeros((ncls_p - NCLS, NHID), np.float32)], 0)
    wcT = np.ascontiguousarray(
        Wc_p.reshape(ncls_p, KCH, 128).transpose(2, 1, 0)
            .reshape(128, KCH * ncls_p).astype(ndt))

    Ww_pad = np.zeros((NCLS_PAD, CHUNK, NHID), np.float32)
    Ww_pad[:NCLS] = Ww

    in_maps = []
    for k in range(NCORES):
        # per-slot k-major weights: tmp[s, j, p, w] = Ww[cls_s, w, j*128+p]
        wsel = Ww_pad[np.maximum(slot_cls[k], 0)]
        wsel[slot_cls[k] < 0] = 0.0
        tmp = wsel.reshape(slots, CHUNK, KCH, 128).transpose(0, 2, 3, 1)
        if gs == 2:
            # group = m-tile (per_mt slots); within: pair r, then j, then
            # the two slots' CHUNK columns side by side
            tmp = tmp.reshape(n_mt, per_mt // 2, 2, KCH, 128, CHUNK)
            tmp = tmp.transpose(0, 4, 1, 3, 2, 5)  # [n_mt,128,pair,j,2,CHUNK]
        else:
            tmp = tmp.reshape(n_mt, per_mt, KCH, 128, CHUNK)
            tmp = tmp.transpose(0, 3, 1, 2, 4)     # [n_mt,128,q,j,CHUNK]
        tmp = tmp.reshape(n_mt, 128, per_mt * KCH * CHUNK)
        if fp8:
            wwT = np.ascontiguousarray(
                (tmp * WSCALE).astype(ml_dtypes.float8_e3m4))
        else:
            wwT = np.ascontiguousarray(tmp.astype(ndt))

        ti = tok_idx[k]
        xk = x[np.maximum(ti, 0)]
        xk[ti < 0] = 0.0
        # xT[p, (m*KCH+j)*128 + t] = xk[m*128+t, j*128+p]
        xT = np.ascontiguousarray(
            xk.reshape(n_mt, 128, KCH, 128).transpose(3, 0, 2, 1)
              .reshape(128, n_mt * KCH * 128).astype(ndt))
        in_maps.append({"xT": xT, "wcT": wcT, "wwT": wwT})

    LAST_RESULT = run_bass_kernel_spmd(
        nc, in_maps, list(range(NCORES)), trace=_trace,
        trace_cores=(_trace_cores if _trace else None))

    wide = block and C == 16
    out = np.zeros((N, NCOL), np.float32)
    if wide:
        # row r of a core's output holds its pair's full 2*CHUNK block;
        # slot parity selects which CHUNK half is this row's class
        a_row = (np.arange(npad) // C) % 2
    for k in range(NCORES):
        ok = np.asarray(LAST_RESULT.results[k]["out"]).astype(np.float32)
        if fp8:
            ok[:, NCLS:] *= 1.0 / WSCALE   # undo host prescale of Ww
        if wide:
            words = np.where((a_row == 0)[:, None],
                             ok[:, NCLS:NCLS + CHUNK],
                             ok[:, NCLS + CHUNK:NCLS + 2 * CHUNK])
            ok = np.concatenate([ok[:, :NCLS], words], 1)
        valid = tok_idx[k] >= 0
        out[tok_idx[k][valid]] = ok[valid]

    if overflow.size:
        # rare capacity-overflow tokens: evaluate directly on the host
        xo = x[overflow]                                   # [no, NHID]
        out[overflow, :NCLS] = xo @ Wc.T
        co = cls[overflow]
        out[overflow, NCLS:] = np.einsum(
            "nkh,nh->nk", Ww[co], xo, optimize=True)

    out[:, :NCLS] += bc
    out[:, NCLS:] += bw[cls]
    return out

